# revision 63
# baseline (speedup 1.0000x reference)
"""3-layer GCN + linear head on 8 Trainium2 NeuronCores.

Sharding: nodes are partitioned across the 8 cores (graph parallel), after a
host-side balanced permutation that gives every 128-node block exactly the
same number of incoming edges (including self loops).  Per layer each core:
  - transforms its 1024 rows (dense matmul, weights replicated),
  - AllGathers the transformed rows to every core,
  - gathers edge-source rows with SWDGE dma_gather and reduces them into
    destination rows with TensorE matmuls against host-built per-chunk
    selection matrices S (which carry the GCN edge normalization weights).
Layer 1 aggregates x first (256-dim) and transforms after, which is cheaper.
All arithmetic is fp32; accumulation in PSUM.
"""
import sys

if "/opt/trn_rl_repo" not in sys.path:
    sys.path.insert(0, "/opt/trn_rl_repo")

import numpy as np

import concourse.bass as bass
import concourse.mybir as mybir
import concourse.tile as tile
from concourse import bacc
from concourse.bass_utils import run_bass_kernel_spmd
from concourse.library_config import mlp

N = 8192
NUM_CORES = 8
R = N // NUM_CORES          # rows per core
NB = 8                      # dst blocks per core (128 rows each)
NBINS = NUM_CORES * NB
BIN_SZ = 128
F_IN, H1, H2, H3, F_OUT = 256, 2048, 2048, 1024, 768
DT = mybir.dt.float32
MM_DT = mybir.dt.bfloat16
F8 = mybir.dt.float8e4
TANH = mybir.ActivationFunctionType.Tanh


# ----------------------------------------------------------------------------
# Host-side graph preprocessing
# ----------------------------------------------------------------------------

def _preprocess(edge_index):
    src = np.asarray(edge_index[0], dtype=np.int64)
    dst = np.asarray(edge_index[1], dtype=np.int64)

    deg = np.bincount(dst, minlength=N).astype(np.float64) + 1.0
    dinv = 1.0 / np.sqrt(deg)
    d_in = np.bincount(dst, minlength=N) + 1

    # greedy balanced partition of nodes into bins of 128, equal in-edge sums
    order = np.argsort(-d_in, kind="stable")
    bin_sum = np.zeros(NBINS, dtype=np.int64)
    bin_cnt = np.zeros(NBINS, dtype=np.int64)
    bin_nodes = [[] for _ in range(NBINS)]
    for node in order:
        avail = np.where(bin_cnt < BIN_SZ)[0]
        b = avail[np.argmin(bin_sum[avail])]
        bin_nodes[b].append(node)
        bin_sum[b] += d_in[node]
        bin_cnt[b] += 1

    target = int(np.ceil(d_in.sum() / NBINS))
    for _ in range(200):
        hi = int(np.argmax(bin_sum))
        if bin_sum[hi] <= target:
            break
        lo = int(np.argmin(bin_sum))
        need = bin_sum[hi] - target
        best = None
        for ai, a in enumerate(bin_nodes[hi]):
            for bi, b in enumerate(bin_nodes[lo]):
                diff = d_in[a] - d_in[b]
                if diff > 0:
                    score = abs(diff - need)
                    if best is None or score < best[0]:
                        best = (score, ai, bi)
        if best is None:
            break
        _, ai, bi = best
        a, b = bin_nodes[hi][ai], bin_nodes[lo][bi]
        bin_nodes[hi][ai], bin_nodes[lo][bi] = b, a
        bin_sum[hi] += d_in[b] - d_in[a]
        bin_sum[lo] += d_in[a] - d_in[b]

    CH = int(np.ceil(bin_sum.max() / 128))
    EPB = CH * 128

    perm = np.concatenate([np.array(bn, dtype=np.int64) for bn in bin_nodes])
    inv = np.empty(N, dtype=np.int64)
    inv[perm] = np.arange(N)

    all_src = np.concatenate([inv[src], np.arange(N, dtype=np.int64)])
    all_dst = np.concatenate([inv[dst], np.arange(N, dtype=np.int64)])
    all_w = np.concatenate([
        (dinv[src] * dinv[dst]).astype(np.float32),
        (dinv[perm] * dinv[perm]).astype(np.float32),
    ])
    # self-loops (the appended tail) are handled as a per-bin diagonal matmul
    # against the core's own (local, pre-AllGather) rows — real edges only in
    # the gather tables.  Layer 1's dense-A keeps the full list.
    is_real = np.zeros(len(all_src), bool)
    is_real[:len(src)] = True
    sl_w = all_w[len(src):]          # dinv^2 per permuted node, in perm order

    bin_of = all_dst // BIN_SZ
    dst_local = all_dst % BIN_SZ

    NKC = N // 128                      # 64 source chunks for dense-A layer 1
    # Source rows are AllGathered in two row-halves per core: half h of core c
    # holds permuted rows [c*R + h*R/2, c*R + (h+1)*R/2), stored at row
    # c*R/2 + (local % (R/2)) of half-tensor h.  Aggregation chunks are
    # partitioned by source half so every dma_gather reads one tensor.
    RH = R // 2
    src_half = (all_src % R) // RH
    src_row = (all_src // R) * RH + (all_src % RH)

    # SPMD: one program for all cores, so chunk counts per (bin-slot, half)
    # must be uniform — take the max need across cores and pad.
    sels = [[np.where(bin_of == c * NB + blk)[0] for blk in range(NB)]
            for c in range(NUM_CORES)]
    rsels = [[s[is_real[s]] for s in row] for row in sels]
    ch_counts = []
    for blk in range(NB):
        nch = []
        for h in range(2):
            need = max(
                int(np.sum(src_half[rsels[c][blk]] == h))
                for c in range(NUM_CORES))
            nch.append(max(1, -(-need // 128)))
        ch_counts.append(tuple(nch))

    dinv_perm = dinv[perm].astype(np.float32)
    idx_tabs, s_mats, a_mats, dv_mats = [], [], [], []
    for c in range(NUM_CORES):
        a_mat = np.zeros((128, NB * NKC, 128), np.float32)
        dv = np.empty((128, NB), np.float32)
        idx_cols, s_cols = [], []
        for blk in range(NB):
            sel = rsels[c][blk]
            nodes = np.arange((c * NB + blk) * 128, (c * NB + blk + 1) * 128)
            dv[:, blk] = dinv_perm[nodes]
            for h in range(2):
                selh = sel[src_half[sel] == h]
                n_e = len(selh)
                n_ch = ch_counts[blk][h]
                epb = n_ch * 128
                e_src = np.zeros(epb, np.int64)
                e_dst = np.zeros(epb, np.int64)
                e_w = np.zeros(epb, np.float32)
                e_src[:n_e] = src_row[selh]
                e_dst[:n_e] = dst_local[selh]
                e_w[:n_e] = all_w[selh]
                ar = np.arange(epb)
                it = np.zeros((16, n_ch * 8), np.int16)
                it[ar % 16, ar // 16] = e_src.astype(np.int16)
                # edge weights are folded into the stored rows (dinv_src) and
                # the post-aggregation scale (dinv_dst): S is a 0/1 selector,
                # exact in fp8
                sm = np.zeros((128, n_ch, 128), np.float32)
                sm[ar % 128, ar // 128, e_dst] = (e_w != 0).astype(np.float32)
                idx_cols.append(it)
                s_cols.append(sm)
            # dense (A+I)-block rows for layer 1 (+= folds duplicate edges);
            # full edge list including self-loops
            fsel = sels[c][blk]
            np.add.at(a_mat,
                      (all_src[fsel] % 128,
                       blk * NKC + all_src[fsel] // 128,
                       dst_local[fsel]),
                      all_w[fsel])
        idx16 = np.concatenate(idx_cols, axis=1)
        idx_tab = np.tile(idx16, (8, 1))
        s_mat = np.concatenate(s_cols, axis=1)
        idx_tabs.append(idx_tab)
        s_mats.append(s_mat)
        a_mats.append(a_mat)
        dv_mats.append(dv)

    return perm, ch_counts, idx_tabs, s_mats, a_mats, dv_mats


# ----------------------------------------------------------------------------
# Device program
# ----------------------------------------------------------------------------

def _build_program(ch_counts, repeat=1, phase_reps=None, mm_dt=DT,
                   no_cc=False):
    """Build the SPMD program.  `repeat` repeats the whole pipeline (for
    timing); `phase_reps` maps phase name -> extra repetitions (timing only:
    a repeated phase recomputes the same values)."""
    pr = dict(L1=1, L2T=1, AG2=1, L2A=1, L3T=1, AG3=1, L3A=1, FIN=1)
    if phase_reps:
        pr.update(phase_reps)
    TOT_CH = sum(h0 + h1 for h0, h1 in ch_counts)
    # chunk-column offset of (blk, h) in the concatenated idx/s layout
    ch_off = []
    off = 0
    for h0, h1 in ch_counts:
        ch_off.append((off, off + h0))
        off += h0 + h1
    RH = R // 2
    nc = bacc.Bacc("TRN2", target_bir_lowering=False, debug=False,
                   num_devices=NUM_CORES)
    core_ids = list(range(NUM_CORES))

    NKC = N // 128
    x_perm = nc.dram_tensor("x_perm", [128, NKC, F_IN], mm_dt,
                            kind="ExternalInput")
    idx_in = nc.dram_tensor("idx_in", [128, TOT_CH * 8], mybir.dt.int16,
                            kind="ExternalInput")
    s_in = nc.dram_tensor("s_in", [128, TOT_CH, 128], F8, kind="ExternalInput")
    a_in = nc.dram_tensor("a_in", [128, NB * NKC, 128], mm_dt,
                          kind="ExternalInput")
    dv_in = nc.dram_tensor("dv_in", [128, NB], DT, kind="ExternalInput")
    ident_in = nc.dram_tensor("ident", [128, 128], mm_dt, kind="ExternalInput")
    # weights pre-tiled on host to [128, K/128, F] layout
    w1t_in = nc.dram_tensor("w1t", [128, F_IN // 128, H1], mm_dt, kind="ExternalInput")
    w2t_in = nc.dram_tensor("w2t", [128, H1 // 128, H2], mm_dt, kind="ExternalInput")
    w3t_in = nc.dram_tensor("w3t", [128, H2 // 128, H3], mm_dt, kind="ExternalInput")
    wlt_in = nc.dram_tensor("wlt", [128, H3 // 128, F_OUT], mm_dt, kind="ExternalInput")
    b1_in = nc.dram_tensor("b1pp", [128, H1 // 128], DT, kind="ExternalInput")
    b2_in = nc.dram_tensor("b2pp", [128, H2 // 128], DT, kind="ExternalInput")
    b3_in = nc.dram_tensor("b3pp", [128, H3 // 128], DT, kind="ExternalInput")
    bl_in = nc.dram_tensor("blb", [128, F_OUT], DT, kind="ExternalInput")

    out = nc.dram_tensor("out", [R, F_OUT], DT, kind="ExternalOutput")

    xw2_locs = [nc.dram_tensor(f"xw2_loc{h}", [RH, H2], mm_dt)
                for h in range(2)]
    xw2_locs8 = [nc.dram_tensor(f"xw2_loc8{h}", [RH, H2], F8)
                 for h in range(2)]
    xw2_fulls = [nc.dram_tensor(f"xw2_full{h}", [N // 2, H2], F8,
                                addr_space="Shared") for h in range(2)]
    xw3_locs = [nc.dram_tensor(f"xw3_loc{h}", [RH, H3], mm_dt)
                for h in range(2)]
    xw3_locs8 = [nc.dram_tensor(f"xw3_loc8{h}", [RH, H3], F8)
                 for h in range(2)]
    xw3_fulls = [nc.dram_tensor(f"xw3_full{h}", [N // 2, H3], F8,
                                addr_space="Shared") for h in range(2)]

    uid = [0]

    def pname(base):
        uid[0] += 1
        return f"{base}{uid[0]}"

    with tile.TileContext(nc) as tc:
        with tc.tile_pool(name="const", bufs=1) as cpool:
            nc.gpsimd.load_library(mlp)
            idx_sb = cpool.tile([128, TOT_CH * 8], mybir.dt.int16, tag="idx")
            nc.sync.dma_start(idx_sb[:], idx_in[:])
            s_sb = cpool.tile([128, TOT_CH, 128], F8, tag="s")
            nc.sync.dma_start(s_sb[:], s_in[:])
            dv_sb = cpool.tile([128, NB], DT, tag="dv")
            nc.sync.dma_start(dv_sb[:], dv_in[:])
            id_sb = cpool.tile([128, 128], mm_dt, tag="ident")
            nc.sync.dma_start(id_sb[:], ident_in[:])
            b1_sb = cpool.tile([128, H1 // 128], DT, tag="b1")
            nc.sync.dma_start(b1_sb[:], b1_in[:])
            b2_sb = cpool.tile([128, H2 // 128], DT, tag="b2")
            nc.sync.dma_start(b2_sb[:], b2_in[:])
            b3_sb = cpool.tile([128, H3 // 128], DT, tag="b3")
            nc.sync.dma_start(b3_sb[:], b3_in[:])
            bl_sb = cpool.tile([128, F_OUT], DT, tag="bl")
            nc.sync.dma_start(bl_sb[:], bl_in[:])

            def phase_L1_L2T():
                """fully per-bin pipeline: dense-A aggregate of x -> transpose
                -> L1 transform+tanh (feature-major per bin) -> L2 transform
                (node-major, dinv-scaled dual write) -> row-half AllGathers
                triggered mid-phase.  No global L1/L2T barrier, so the PE
                never drains and AG2-h0 overlaps bins 4-7."""
                with (
                    tc.tile_pool(name=pname("f12"), bufs=1) as fpool,
                    tc.tile_pool(name=pname("f12ps"), bufs=1, space="PSUM") as fps,
                ):
                    w1t_sb = fpool.tile([128, F_IN // 128, H1], mm_dt, tag="w1t")
                    nc.sync.dma_start(w1t_sb[:], w1t_in[:])
                    w2t_sb = fpool.tile([128, H1 // 128, H2], mm_dt, tag="w2t")
                    nc.sync.dma_start(w2t_sb[:], w2t_in[:])
                    x_sb = fpool.tile([128, NKC, F_IN], mm_dt, tag="xall")
                    nc.sync.dma_start(x_sb[:], x_perm[:])
                    H1H = H1 // 2
                    for blk in range(NB):
                        a_sb = fpool.tile([128, NKC, 128], mm_dt, tag="a1",
                                          bufs=2)
                        nc.sync.dma_start(
                            a_sb[:], a_in[:, blk * NKC:(blk + 1) * NKC, :])
                        ps = fps.tile([128, F_IN], DT, tag="agg", bufs=2)
                        for k in range(NKC):
                            nc.tensor.matmul(
                                ps[:, :], a_sb[:, k, :], x_sb[:, k, :],
                                start=(k == 0), stop=(k == NKC - 1))
                        a_nm = fpool.tile([128, F_IN], mm_dt, tag="anm", bufs=2)
                        nc.vector.tensor_copy(a_nm[:], ps[:])
                        agg_t = fpool.tile([128, F_IN // 128, 128], mm_dt,
                                           tag="aggt", bufs=2)
                        for f in range(F_IN // 128):
                            pt = fps.tile([128, 128], mm_dt, tag="pt", bufs=2)
                            nc.tensor.transpose(
                                pt[:], a_nm[:, f * 128:(f + 1) * 128], id_sb[:])
                            nc.vector.tensor_copy(agg_t[:, f, :], pt[:])
                        h1r = fpool.tile([128, H1 // 128, 128], mm_dt,
                                         tag="h1r", bufs=2)
                        for hh in range(2):
                            ps1 = fps.tile([128, H1H], DT, tag="xw1", bufs=1)
                            for k in range(F_IN // 128):
                                for n0 in range(0, H1H, 512):
                                    nc.tensor.matmul(
                                        ps1[:, n0:n0 + 512],
                                        agg_t[:, k, :],
                                        w1t_sb[:, k, hh * H1H + n0:
                                               hh * H1H + n0 + 512],
                                        start=(k == 0),
                                        stop=(k == F_IN // 128 - 1))
                            o1 = fpool.tile([128, H1H], mm_dt, tag="o1",
                                            bufs=2)
                            nc.vector.tensor_copy(o1[:], ps1[:])
                            for f in range(H1H // 128):
                                fg = hh * (H1H // 128) + f
                                pt = fps.tile([128, 128], mm_dt, tag="pt",
                                              bufs=2)
                                nc.tensor.transpose(
                                    pt[:], o1[:, f * 128:(f + 1) * 128],
                                    id_sb[:])
                                nc.scalar.activation(
                                    h1r[:, fg, :], pt[:], TANH,
                                    bias=b1_sb[:, fg:fg + 1])
                        hb, rb = blk // (NB // 2), blk % (NB // 2)
                        for hh2 in range(2):
                            c0 = hh2 * (H2 // 2)
                            ps2 = fps.tile([128, H2 // 2], DT, tag="xw2",
                                           bufs=1)
                            for k in range(H1 // 128):
                                for n0 in range(0, H2 // 2, 512):
                                    nc.tensor.matmul(
                                        ps2[:, n0:n0 + 512],
                                        h1r[:, k, :],
                                        w2t_sb[:, k, c0 + n0:c0 + n0 + 512],
                                        start=(k == 0),
                                        stop=(k == H1 // 128 - 1))
                            o = fpool.tile([128, H2 // 2], mm_dt, tag="o",
                                           bufs=2)
                            nc.vector.tensor_scalar_mul(
                                o[:], ps2[:], dv_sb[:, blk:blk + 1])
                            nc.sync.dma_start(
                                xw2_locs[hb][rb * 128:(rb + 1) * 128,
                                             c0:c0 + H2 // 2], o[:])
                            o8 = fpool.tile([128, H2 // 2], F8, tag="o8",
                                            bufs=2)
                            nc.vector.tensor_scalar_mul(
                                o8[:], ps2[:], dv_sb[:, blk:blk + 1])
                            nc.sync.dma_start(
                                xw2_locs8[hb][rb * 128:(rb + 1) * 128,
                                              c0:c0 + H2 // 2], o8[:])
                        if rb == NB // 2 - 1:
                            if no_cc:
                                for cb in range(NUM_CORES):
                                    nc.sync.dma_start(
                                        xw2_fulls[hb][cb * RH:(cb + 1) * RH, :],
                                        xw2_locs8[hb][:])
                            else:
                                nc.gpsimd.collective_compute(
                                    "AllGather", mybir.AluOpType.bypass,
                                    replica_groups=[core_ids],
                                    ins=[xw2_locs8[hb][:]],
                                    outs=[xw2_fulls[hb][:]])

            def transform(ht, KD, FD, wt_in, locs, locs8, fulls):
                """full-width (ht rows) @ W^T per 128-row block, node-major,
                pre-scaled by dinv of the row's node; written twice: bf16 for
                the local self-loop path and fp8 for the AllGather + gather
                path.  Each ROW half AllGathers as soon as its 4 blocks are
                written so the collective overlaps the other half's matmuls."""
                HK = KD // 128
                with (
                    tc.tile_pool(name=pname("tr"), bufs=1) as tpool,
                    tc.tile_pool(name=pname("trps"), bufs=1, space="PSUM") as tps,
                ):
                    wt_sb = tpool.tile([128, HK, FD], mm_dt, tag="wt")
                    nc.sync.dma_start(wt_sb[:], wt_in[:])
                    for h in range(2):
                        for r2 in range(NB // 2):
                            r = h * (NB // 2) + r2
                            ps = tps.tile([128, FD], DT, tag="xw", bufs=2)
                            for k in range(HK):
                                for n0 in range(0, FD, 512):
                                    n1 = min(n0 + 512, FD)
                                    nc.tensor.matmul(
                                        ps[:, n0:n1],
                                        ht[:, k, r * 128:(r + 1) * 128],
                                        wt_sb[:, k, n0:n1],
                                        start=(k == 0), stop=(k == HK - 1))
                            o = tpool.tile([128, FD], mm_dt, tag="o", bufs=3)
                            nc.vector.tensor_scalar_mul(
                                o[:], ps[:], dv_sb[:, r:r + 1])
                            nc.sync.dma_start(
                                locs[h][r2 * 128:(r2 + 1) * 128, :], o[:])
                            o8 = tpool.tile([128, FD], F8, tag="o8", bufs=3)
                            nc.vector.tensor_scalar_mul(
                                o8[:], ps[:], dv_sb[:, r:r + 1])
                            nc.sync.dma_start(
                                locs8[h][r2 * 128:(r2 + 1) * 128, :], o8[:])
                        if no_cc:
                            for cb in range(NUM_CORES):
                                nc.sync.dma_start(
                                    fulls[h][cb * RH:(cb + 1) * RH, :],
                                    locs8[h][:])
                        else:
                            nc.gpsimd.collective_compute(
                                "AllGather", mybir.AluOpType.bypass,
                                replica_groups=[core_ids],
                                ins=[locs8[h][:]], outs=[fulls[h][:]])

            def aggregate(fulls, locs, FD, ht, b_sb, setup_fn=None,
                          consumer_fn=None):
                """gather full-width rows by edge source (chunks partitioned
                by source row-half so each gather reads one contiguous
                AllGathered tensor), node-major reduce, transpose +
                tanh(.+bias) into feature-major ht.  Self-loops are a
                diagonal matmul against the core's own local rows (no
                collective dependency).  consumer_fn(blk, state, pools) emits
                the next phase's work for bin blk right after its activation,
                filling PE bubbles during later bins' gathers."""
                with (
                    tc.tile_pool(name=pname("ag"), bufs=1) as apool,
                    tc.tile_pool(name=pname("agps"), bufs=1, space="PSUM") as aps,
                ):
                    agg_bufs = 1 if FD * 4 // 512 >= 8 else 2
                    GB = max(1, 10240 // FD)
                    state = setup_fn(apool) if setup_fn else None
                    for i in range(2):
                        gz = apool.tile([128, GB, FD], F8, tag="g", bufs=2)
                        nc.vector.memset(gz[:], 0)
                    for blk in range(NB):
                        ps = aps.tile([128, FD], DT, tag="agg", bufs=agg_bufs)
                        hb, rb = blk // (NB // 2), blk % (NB // 2)
                        sl = apool.tile([128, FD], mm_dt, tag="sl", bufs=2)
                        nc.sync.dma_start(
                            sl[:], locs[hb][rb * 128:(rb + 1) * 128, :])
                        for nf in range(FD // 512):
                            nc.tensor.matmul(
                                ps[:, nf * 512:(nf + 1) * 512],
                                id_sb[:],
                                sl[:, nf * 512:(nf + 1) * 512],
                                start=True, stop=False)
                        tot = ch_counts[blk][0] + ch_counts[blk][1]
                        done = 0
                        for h in range(2):
                            nch = ch_counts[blk][h]
                            base = ch_off[blk][h]
                            for c0 in range(0, nch, GB):
                                gb = min(GB, nch - c0)
                                t0 = base + c0
                                g = apool.tile([128, GB, FD], F8,
                                               tag="g", bufs=2)
                                nc.gpsimd.dma_gather(
                                    g[:, :gb, :], fulls[h][:],
                                    idx_sb[:, t0 * 8:(t0 + gb) * 8],
                                    gb * 128, gb * 128, FD,
                                    single_packet=False)
                                for j in range(gb):
                                    for nf in range(FD // 512):
                                        nc.tensor.matmul(
                                            ps[:, nf * 512:(nf + 1) * 512],
                                            s_sb[:, t0 + j, :],
                                            g[:, j, nf * 512:(nf + 1) * 512],
                                            start=False,
                                            stop=(done == tot - 1))
                                    done += 1
                        a_nm = apool.tile([128, FD], mm_dt, tag="anm", bufs=2)
                        nc.vector.tensor_scalar_mul(
                            a_nm[:], ps[:], dv_sb[:, blk:blk + 1])
                        for f in range(FD // 128):
                            pt = aps.tile([128, 128], mm_dt, tag="pt", bufs=2)
                            nc.tensor.transpose(
                                pt[:], a_nm[:, f * 128:(f + 1) * 128], id_sb[:])
                            nc.scalar.activation(
                                ht[:, f, blk * 128:(blk + 1) * 128], pt[:],
                                TANH, bias=b_sb[:, f:f + 1])
                        if consumer_fn:
                            consumer_fn(blk, state, apool, aps)

            for rep in range(repeat):
                for _ in range(pr["L1"]):
                    phase_L1_L2T()
                with tc.tile_pool(name=pname("h2t"), bufs=1) as h2t_pool:
                    h2t = h2t_pool.tile([128, H2 // 128, R], mm_dt, tag="h2t")

                    def l3t_setup(apool):
                        wt3_sb = apool.tile([128, H2 // 128, H3], mm_dt,
                                            tag="wt3")
                        nc.sync.dma_start(wt3_sb[:], w3t_in[:])
                        return wt3_sb

                    def l3t_consumer(blk, wt3_sb, apool, aps):
                        """transform bin blk of h2t into xw3 rows; AllGather
                        each row half as soon as its 4 bins are done."""
                        ps3 = aps.tile([128, H3], DT, tag="xw3", bufs=1)
                        for k in range(H2 // 128):
                            for n0 in range(0, H3, 512):
                                n1 = min(n0 + 512, H3)
                                nc.tensor.matmul(
                                    ps3[:, n0:n1],
                                    h2t[:, k, blk * 128:(blk + 1) * 128],
                                    wt3_sb[:, k, n0:n1],
                                    start=(k == 0),
                                    stop=(k == H2 // 128 - 1))
                        o3 = apool.tile([128, H3], mm_dt, tag="o3", bufs=2)
                        nc.vector.tensor_scalar_mul(
                            o3[:], ps3[:], dv_sb[:, blk:blk + 1])
                        hb, rb = blk // (NB // 2), blk % (NB // 2)
                        nc.sync.dma_start(
                            xw3_locs[hb][rb * 128:(rb + 1) * 128, :], o3[:])
                        o38 = apool.tile([128, H3], F8, tag="o38", bufs=2)
                        nc.vector.tensor_scalar_mul(
                            o38[:], ps3[:], dv_sb[:, blk:blk + 1])
                        nc.sync.dma_start(
                            xw3_locs8[hb][rb * 128:(rb + 1) * 128, :], o38[:])
                        if rb == NB // 2 - 1:
                            if no_cc:
                                for cb in range(NUM_CORES):
                                    nc.sync.dma_start(
                                        xw3_fulls[hb][cb * RH:(cb + 1) * RH, :],
                                        xw3_locs8[hb][:])
                            else:
                                nc.gpsimd.collective_compute(
                                    "AllGather", mybir.AluOpType.bypass,
                                    replica_groups=[core_ids],
                                    ins=[xw3_locs8[hb][:]],
                                    outs=[xw3_fulls[hb][:]])

                    for _ in range(pr["L2A"]):
                        aggregate(xw2_fulls, xw2_locs, H2, h2t, b2_sb,
                                  setup_fn=l3t_setup,
                                  consumer_fn=l3t_consumer)
                with tc.tile_pool(name=pname("h3t"), bufs=1) as h3t_pool:
                    h3t = h3t_pool.tile([128, H3 // 128, R], mm_dt, tag="h3t")

                    def fin_setup(apool):
                        wlt_sb = apool.tile([128, H3 // 128, F_OUT], mm_dt,
                                            tag="wlt")
                        nc.sync.dma_start(wlt_sb[:], wlt_in[:])
                        return wlt_sb

                    def fin_consumer(blk, wlt_sb, apool, aps):
                        psf = aps.tile([128, F_OUT], DT, tag="xwf", bufs=1)
                        for k in range(H3 // 128):
                            for n0 in range(0, F_OUT, 512):
                                n1 = min(n0 + 512, F_OUT)
                                nc.tensor.matmul(
                                    psf[:, n0:n1],
                                    h3t[:, k, blk * 128:(blk + 1) * 128],
                                    wlt_sb[:, k, n0:n1],
                                    start=(k == 0),
                                    stop=(k == H3 // 128 - 1))
                        o = apool.tile([128, F_OUT], DT, tag="of", bufs=2)
                        nc.vector.tensor_tensor(
                            out=o[:], in0=psf[:], in1=bl_sb[:],
                            op=mybir.AluOpType.add)
                        nc.sync.dma_start(
                            out[blk * 128:(blk + 1) * 128, :], o[:])

                    for _ in range(pr["L3A"]):
                        aggregate(xw3_fulls, xw3_locs, H3, h3t, b3_sb,
                                  setup_fn=fin_setup,
                                  consumer_fn=fin_consumer)

    nc.compile()
    return nc


# ----------------------------------------------------------------------------
# Entry point
# ----------------------------------------------------------------------------

def _make_in_maps(inputs, perm, idx_tabs, s_mats, a_mats, dv_mats):
    import ml_dtypes
    mm_np = ml_dtypes.bfloat16 if MM_DT == mybir.dt.bfloat16 else np.float32
    f8_np = ml_dtypes.float8_e4m3

    def tile_w(w):  # [K, F] -> [128, K/128, F]
        k, f = w.shape
        return np.ascontiguousarray(
            w.reshape(k // 128, 128, f).transpose(1, 0, 2)).astype(mm_np)

    x_perm = np.ascontiguousarray(
        np.asarray(inputs["x"], np.float32)[perm]
        .reshape(N // 128, 128, F_IN).transpose(1, 0, 2)).astype(mm_np)
    w1t = tile_w(np.ascontiguousarray(np.asarray(inputs["W1"], np.float32).T))
    w2t = tile_w(np.ascontiguousarray(np.asarray(inputs["W2"], np.float32).T))
    w3t = tile_w(np.ascontiguousarray(np.asarray(inputs["W3"], np.float32).T))
    wlt = tile_w(np.ascontiguousarray(np.asarray(inputs["Wl"], np.float32).T))
    b1pp = np.ascontiguousarray(
        np.asarray(inputs["b1"], np.float32).reshape(-1, 128).T)
    b2pp = np.ascontiguousarray(
        np.asarray(inputs["b2"], np.float32).reshape(-1, 128).T)
    b3pp = np.ascontiguousarray(
        np.asarray(inputs["b3"], np.float32).reshape(-1, 128).T)
    blb = np.ascontiguousarray(
        np.broadcast_to(np.asarray(inputs["bl"], np.float32), (128, F_OUT)))
    ident = np.eye(128, dtype=mm_np)

    in_maps = []
    for c in range(NUM_CORES):
        in_maps.append({
            "x_perm": x_perm, "idx_in": idx_tabs[c],
            "s_in": s_mats[c].astype(f8_np),
            "a_in": a_mats[c].astype(mm_np),
            "dv_in": dv_mats[c],
            "ident": ident,
            "w1t": w1t, "w2t": w2t, "w3t": w3t, "wlt": wlt,
            "b1pp": b1pp, "b2pp": b2pp, "b3pp": b3pp, "blb": blb,
        })
    return in_maps


def _run(inputs, trace=False):
    perm, ch_counts, idx_tabs, s_mats, a_mats, dv_mats = _preprocess(
        np.asarray(inputs["edge_index"]))
    nc = _build_program(ch_counts, mm_dt=MM_DT)
    in_maps = _make_in_maps(inputs, perm, idx_tabs, s_mats, a_mats, dv_mats)
    res = run_bass_kernel_spmd(nc, in_maps, list(range(NUM_CORES)), trace=trace)
    out_perm = np.concatenate([res.results[c]["out"] for c in range(NUM_CORES)], 0)
    out = np.empty_like(out_perm)
    out[perm] = out_perm
    return out, res


def kernel(**inputs):
    out, _ = _run(inputs, trace=False)
    return out



# revision 67
# speedup vs baseline: 1.0369x; 1.0369x over previous
"""3-layer GCN + linear head on 8 Trainium2 NeuronCores.

Sharding: nodes are partitioned across the 8 cores (graph parallel), after a
host-side balanced permutation that gives every 128-node block exactly the
same number of incoming edges (including self loops).  Per layer each core:
  - transforms its 1024 rows (dense matmul, weights replicated),
  - AllGathers the transformed rows to every core,
  - gathers edge-source rows with SWDGE dma_gather and reduces them into
    destination rows with TensorE matmuls against host-built per-chunk
    selection matrices S (which carry the GCN edge normalization weights).
Layer 1 aggregates x first (256-dim) and transforms after, which is cheaper.
All arithmetic is fp32; accumulation in PSUM.
"""
import sys

if "/opt/trn_rl_repo" not in sys.path:
    sys.path.insert(0, "/opt/trn_rl_repo")

import numpy as np

import concourse.bass as bass
import concourse.mybir as mybir
import concourse.tile as tile
from concourse import bacc
from concourse.bass_utils import run_bass_kernel_spmd
from concourse.library_config import mlp

N = 8192
NUM_CORES = 8
R = N // NUM_CORES          # rows per core
NB = 8                      # dst blocks per core (128 rows each)
NBINS = NUM_CORES * NB
BIN_SZ = 128
F_IN, H1, H2, H3, F_OUT = 256, 2048, 2048, 1024, 768
DT = mybir.dt.float32
MM_DT = mybir.dt.bfloat16
F8 = mybir.dt.float8e4
TANH = mybir.ActivationFunctionType.Tanh


# ----------------------------------------------------------------------------
# Host-side graph preprocessing
# ----------------------------------------------------------------------------

def _preprocess(edge_index):
    src = np.asarray(edge_index[0], dtype=np.int64)
    dst = np.asarray(edge_index[1], dtype=np.int64)

    deg = np.bincount(dst, minlength=N).astype(np.float64) + 1.0
    dinv = 1.0 / np.sqrt(deg)
    d_in = np.bincount(dst, minlength=N) + 1

    # greedy balanced partition of nodes into bins of 128, equal in-edge sums
    order = np.argsort(-d_in, kind="stable")
    bin_sum = np.zeros(NBINS, dtype=np.int64)
    bin_cnt = np.zeros(NBINS, dtype=np.int64)
    bin_nodes = [[] for _ in range(NBINS)]
    for node in order:
        avail = np.where(bin_cnt < BIN_SZ)[0]
        b = avail[np.argmin(bin_sum[avail])]
        bin_nodes[b].append(node)
        bin_sum[b] += d_in[node]
        bin_cnt[b] += 1

    target = int(np.ceil(d_in.sum() / NBINS))
    for _ in range(200):
        hi = int(np.argmax(bin_sum))
        if bin_sum[hi] <= target:
            break
        lo = int(np.argmin(bin_sum))
        need = bin_sum[hi] - target
        best = None
        for ai, a in enumerate(bin_nodes[hi]):
            for bi, b in enumerate(bin_nodes[lo]):
                diff = d_in[a] - d_in[b]
                if diff > 0:
                    score = abs(diff - need)
                    if best is None or score < best[0]:
                        best = (score, ai, bi)
        if best is None:
            break
        _, ai, bi = best
        a, b = bin_nodes[hi][ai], bin_nodes[lo][bi]
        bin_nodes[hi][ai], bin_nodes[lo][bi] = b, a
        bin_sum[hi] += d_in[b] - d_in[a]
        bin_sum[lo] += d_in[a] - d_in[b]

    CH = int(np.ceil(bin_sum.max() / 128))
    EPB = CH * 128

    perm = np.concatenate([np.array(bn, dtype=np.int64) for bn in bin_nodes])
    inv = np.empty(N, dtype=np.int64)
    inv[perm] = np.arange(N)

    all_src = np.concatenate([inv[src], np.arange(N, dtype=np.int64)])
    all_dst = np.concatenate([inv[dst], np.arange(N, dtype=np.int64)])
    all_w = np.concatenate([
        (dinv[src] * dinv[dst]).astype(np.float32),
        (dinv[perm] * dinv[perm]).astype(np.float32),
    ])
    # self-loops (the appended tail) are handled as a per-bin diagonal matmul
    # against the core's own (local, pre-AllGather) rows — real edges only in
    # the gather tables.  Layer 1's dense-A keeps the full list.
    is_real = np.zeros(len(all_src), bool)
    is_real[:len(src)] = True
    sl_w = all_w[len(src):]          # dinv^2 per permuted node, in perm order

    bin_of = all_dst // BIN_SZ
    dst_local = all_dst % BIN_SZ

    NKC = N // 128                      # 64 source chunks for dense-A layer 1
    # Source rows are AllGathered in two row-halves per core: half h of core c
    # holds permuted rows [c*R + h*R/2, c*R + (h+1)*R/2), stored at row
    # c*R/2 + (local % (R/2)) of half-tensor h.  Aggregation chunks are
    # partitioned by source half so every dma_gather reads one tensor.
    RH = R // 2
    src_half = (all_src % R) // RH
    src_row = (all_src // R) * RH + (all_src % RH)

    # SPMD: one program for all cores, so chunk counts per (bin-slot, half)
    # must be uniform — take the max need across cores and pad.
    sels = [[np.where(bin_of == c * NB + blk)[0] for blk in range(NB)]
            for c in range(NUM_CORES)]
    rsels = [[s[is_real[s]] for s in row] for row in sels]
    ch_counts = []
    for blk in range(NB):
        nch = []
        for h in range(2):
            need = max(
                int(np.sum(src_half[rsels[c][blk]] == h))
                for c in range(NUM_CORES))
            nch.append(max(1, -(-need // 128)))
        ch_counts.append(tuple(nch))

    dinv_perm = dinv[perm].astype(np.float32)
    idx_tabs, s_mats, a_mats, dv_mats = [], [], [], []
    for c in range(NUM_CORES):
        a_mat = np.zeros((128, NB * NKC, 128), np.float32)
        dv = np.empty((128, NB), np.float32)
        idx_cols, s_cols = [], []
        for blk in range(NB):
            sel = rsels[c][blk]
            nodes = np.arange((c * NB + blk) * 128, (c * NB + blk + 1) * 128)
            dv[:, blk] = dinv_perm[nodes]
            for h in range(2):
                selh = sel[src_half[sel] == h]
                n_e = len(selh)
                n_ch = ch_counts[blk][h]
                epb = n_ch * 128
                e_src = np.zeros(epb, np.int64)
                e_dst = np.zeros(epb, np.int64)
                e_w = np.zeros(epb, np.float32)
                e_src[:n_e] = src_row[selh]
                e_dst[:n_e] = dst_local[selh]
                e_w[:n_e] = all_w[selh]
                ar = np.arange(epb)
                it = np.zeros((16, n_ch * 8), np.int16)
                it[ar % 16, ar // 16] = e_src.astype(np.int16)
                # edge weights are folded into the stored rows (dinv_src) and
                # the post-aggregation scale (dinv_dst): S is a 0/1 selector,
                # exact in fp8
                sm = np.zeros((128, n_ch, 128), np.float32)
                sm[ar % 128, ar // 128, e_dst] = (e_w != 0).astype(np.float32)
                idx_cols.append(it)
                s_cols.append(sm)
            # dense (A+I)-block rows for layer 1 (+= folds duplicate edges);
            # full edge list including self-loops
            fsel = sels[c][blk]
            np.add.at(a_mat,
                      (all_src[fsel] % 128,
                       blk * NKC + all_src[fsel] // 128,
                       dst_local[fsel]),
                      all_w[fsel])
        idx16 = np.concatenate(idx_cols, axis=1)
        idx_tab = np.tile(idx16, (8, 1))
        s_mat = np.concatenate(s_cols, axis=1)
        idx_tabs.append(idx_tab)
        s_mats.append(s_mat)
        a_mats.append(a_mat)
        dv_mats.append(dv)

    return perm, ch_counts, idx_tabs, s_mats, a_mats, dv_mats


# ----------------------------------------------------------------------------
# Device program
# ----------------------------------------------------------------------------

def _build_program(ch_counts, repeat=1, phase_reps=None, mm_dt=DT,
                   no_cc=False):
    """Build the SPMD program.  `repeat` repeats the whole pipeline (for
    timing); `phase_reps` maps phase name -> extra repetitions (timing only:
    a repeated phase recomputes the same values)."""
    pr = dict(L1=1, L2T=1, AG2=1, L2A=1, L3T=1, AG3=1, L3A=1, FIN=1)
    if phase_reps:
        pr.update(phase_reps)
    TOT_CH = sum(h0 + h1 for h0, h1 in ch_counts)
    # chunk-column offset of (blk, h) in the concatenated idx/s layout
    ch_off = []
    off = 0
    for h0, h1 in ch_counts:
        ch_off.append((off, off + h0))
        off += h0 + h1
    RH = R // 2
    nc = bacc.Bacc("TRN2", target_bir_lowering=False, debug=False,
                   num_devices=NUM_CORES)
    core_ids = list(range(NUM_CORES))

    NKC = N // 128
    x_perm = nc.dram_tensor("x_perm", [128, NKC, F_IN], mm_dt,
                            kind="ExternalInput")
    idx_in = nc.dram_tensor("idx_in", [128, TOT_CH * 8], mybir.dt.int16,
                            kind="ExternalInput")
    s_in = nc.dram_tensor("s_in", [128, TOT_CH, 128], F8, kind="ExternalInput")
    a_in = nc.dram_tensor("a_in", [128, NB * NKC, 128], mm_dt,
                          kind="ExternalInput")
    dv_in = nc.dram_tensor("dv_in", [128, NB], DT, kind="ExternalInput")
    ident_in = nc.dram_tensor("ident", [128, 128], mm_dt, kind="ExternalInput")
    # weights pre-tiled on host to [128, K/128, F] layout
    w1t_in = nc.dram_tensor("w1t", [128, F_IN // 128, H1], mm_dt, kind="ExternalInput")
    w2t_in = nc.dram_tensor("w2t", [128, H1 // 128, H2], mm_dt, kind="ExternalInput")
    w3t_in = nc.dram_tensor("w3t", [128, H2 // 128, H3], mm_dt, kind="ExternalInput")
    wlt_in = nc.dram_tensor("wlt", [128, H3 // 128, F_OUT], mm_dt, kind="ExternalInput")
    b1_in = nc.dram_tensor("b1pp", [128, H1 // 128], DT, kind="ExternalInput")
    b2_in = nc.dram_tensor("b2pp", [128, H2 // 128], DT, kind="ExternalInput")
    b3_in = nc.dram_tensor("b3pp", [128, H3 // 128], DT, kind="ExternalInput")
    bl_in = nc.dram_tensor("blb", [128, F_OUT], DT, kind="ExternalInput")

    out = nc.dram_tensor("out", [R, F_OUT], DT, kind="ExternalOutput")

    xw2_locs = [nc.dram_tensor(f"xw2_loc{h}", [RH, H2], mm_dt)
                for h in range(2)]
    xw2_locs8 = [nc.dram_tensor(f"xw2_loc8{h}", [RH, H2], F8)
                 for h in range(2)]
    xw2_fulls = [nc.dram_tensor(f"xw2_full{h}", [N // 2, H2], F8,
                                addr_space="Shared") for h in range(2)]
    xw3_locs = [nc.dram_tensor(f"xw3_loc{h}", [RH, H3], mm_dt)
                for h in range(2)]
    xw3_locs8 = [nc.dram_tensor(f"xw3_loc8{h}", [RH, H3], F8)
                 for h in range(2)]
    xw3_fulls = [nc.dram_tensor(f"xw3_full{h}", [N // 2, H3], F8,
                                addr_space="Shared") for h in range(2)]

    uid = [0]

    def pname(base):
        uid[0] += 1
        return f"{base}{uid[0]}"

    with tile.TileContext(nc) as tc:
        with tc.tile_pool(name="const", bufs=1) as cpool:
            nc.gpsimd.load_library(mlp)
            idx_sb = cpool.tile([128, TOT_CH * 8], mybir.dt.int16, tag="idx")
            nc.sync.dma_start(idx_sb[:], idx_in[:])
            s_sb = cpool.tile([128, TOT_CH, 128], F8, tag="s")
            nc.sync.dma_start(s_sb[:], s_in[:])
            dv_sb = cpool.tile([128, NB], DT, tag="dv")
            nc.sync.dma_start(dv_sb[:], dv_in[:])
            id_sb = cpool.tile([128, 128], mm_dt, tag="ident")
            nc.sync.dma_start(id_sb[:], ident_in[:])
            b1_sb = cpool.tile([128, H1 // 128], DT, tag="b1")
            nc.sync.dma_start(b1_sb[:], b1_in[:])
            b2_sb = cpool.tile([128, H2 // 128], DT, tag="b2")
            nc.sync.dma_start(b2_sb[:], b2_in[:])
            b3_sb = cpool.tile([128, H3 // 128], DT, tag="b3")
            nc.sync.dma_start(b3_sb[:], b3_in[:])
            bl_sb = cpool.tile([128, F_OUT], DT, tag="bl")
            nc.sync.dma_start(bl_sb[:], bl_in[:])

            def phase_L1(h1t):
                """dense-A aggregate of x (node-major), transpose, transform."""
                with (
                    tc.tile_pool(name=pname("l1a"), bufs=1) as l1a_pool,
                    tc.tile_pool(name=pname("l1ps"), bufs=1, space="PSUM") as l1ps,
                ):
                    w1t_sb = l1a_pool.tile([128, F_IN // 128, H1], mm_dt, tag="w1t")
                    nc.sync.dma_start(w1t_sb[:], w1t_in[:])
                    x_sb = l1a_pool.tile([128, NKC, F_IN], mm_dt, tag="xall")
                    nc.sync.dma_start(x_sb[:], x_perm[:])
                    agg1t = l1a_pool.tile([128, F_IN // 128, R], mm_dt, tag="agg1t")
                    for blk in range(NB):
                        a_sb = l1a_pool.tile([128, NKC, 128], mm_dt, tag="a1",
                                             bufs=2)
                        nc.sync.dma_start(
                            a_sb[:], a_in[:, blk * NKC:(blk + 1) * NKC, :])
                        ps = l1ps.tile([128, F_IN], DT, tag="agg", bufs=2)
                        for k in range(NKC):
                            nc.tensor.matmul(
                                ps[:, :], a_sb[:, k, :], x_sb[:, k, :],
                                start=(k == 0), stop=(k == NKC - 1))
                        a_nm = l1a_pool.tile([128, F_IN], mm_dt, tag="anm", bufs=2)
                        nc.vector.tensor_copy(a_nm[:], ps[:])
                        for f in range(F_IN // 128):
                            pt = l1ps.tile([128, 128], mm_dt, tag="pt", bufs=2)
                            nc.tensor.transpose(
                                pt[:], a_nm[:, f * 128:(f + 1) * 128], id_sb[:])
                            nc.vector.tensor_copy(
                                agg1t[:, f, blk * 128:(blk + 1) * 128], pt[:])
                    for m in range(H1 // 128):
                        ps = l1ps.tile([128, R], DT, tag="xw", bufs=2)
                        for k in range(F_IN // 128):
                            for n in range(0, R, 512):
                                nc.tensor.matmul(
                                    ps[:, n:n + 512],
                                    w1t_sb[:, k, m * 128:(m + 1) * 128],
                                    agg1t[:, k, n:n + 512],
                                    start=(k == 0), stop=(k == F_IN // 128 - 1))
                        nc.scalar.activation(
                            h1t[:, m, :], ps[:], TANH, bias=b1_sb[:, m:m + 1])

            def transform(ht, KD, FD, wt_in, locs, locs8, fulls):
                """full-width (ht rows) @ W^T per 128-row block, node-major,
                pre-scaled by dinv of the row's node; written twice: bf16 for
                the local self-loop path and fp8 for the AllGather + gather
                path.  Each ROW half AllGathers as soon as its 4 blocks are
                written so the collective overlaps the other half's matmuls."""
                HK = KD // 128
                with (
                    tc.tile_pool(name=pname("tr"), bufs=1) as tpool,
                    tc.tile_pool(name=pname("trps"), bufs=1, space="PSUM") as tps,
                ):
                    wt_sb = tpool.tile([128, HK, FD], mm_dt, tag="wt")
                    nc.sync.dma_start(wt_sb[:], wt_in[:])
                    for h in range(2):
                        for r2 in range(NB // 2):
                            r = h * (NB // 2) + r2
                            ps = tps.tile([128, FD], DT, tag="xw", bufs=2)
                            for k in range(HK):
                                for n0 in range(0, FD, 512):
                                    n1 = min(n0 + 512, FD)
                                    nc.tensor.matmul(
                                        ps[:, n0:n1],
                                        ht[:, k, r * 128:(r + 1) * 128],
                                        wt_sb[:, k, n0:n1],
                                        start=(k == 0), stop=(k == HK - 1))
                            o = tpool.tile([128, FD], mm_dt, tag="o", bufs=3)
                            nc.vector.tensor_scalar_mul(
                                o[:], ps[:], dv_sb[:, r:r + 1])
                            nc.sync.dma_start(
                                locs[h][r2 * 128:(r2 + 1) * 128, :], o[:])
                            o8 = tpool.tile([128, FD], F8, tag="o8", bufs=3)
                            nc.vector.tensor_scalar_mul(
                                o8[:], ps[:], dv_sb[:, r:r + 1])
                            nc.sync.dma_start(
                                locs8[h][r2 * 128:(r2 + 1) * 128, :], o8[:])
                        if no_cc:
                            for cb in range(NUM_CORES):
                                nc.sync.dma_start(
                                    fulls[h][cb * RH:(cb + 1) * RH, :],
                                    locs8[h][:])
                        else:
                            nc.gpsimd.collective_compute(
                                "AllGather", mybir.AluOpType.bypass,
                                replica_groups=[core_ids],
                                ins=[locs8[h][:]], outs=[fulls[h][:]])

            def aggregate(fulls, locs, FD, ht, b_sb, setup_fn=None,
                          consumer_fn=None):
                """gather full-width rows by edge source (chunks partitioned
                by source row-half so each gather reads one contiguous
                AllGathered tensor), node-major reduce, transpose +
                tanh(.+bias) into feature-major ht.  Self-loops are a
                diagonal matmul against the core's own local rows (no
                collective dependency).  consumer_fn(blk, state, pools) emits
                the next phase's work for bin blk right after its activation,
                filling PE bubbles during later bins' gathers."""
                with (
                    tc.tile_pool(name=pname("ag"), bufs=1) as apool,
                    tc.tile_pool(name=pname("agps"), bufs=1, space="PSUM") as aps,
                ):
                    GB = max(1, 10240 // FD)
                    state = setup_fn(apool) if setup_fn else None
                    for i in range(2):
                        gz = apool.tile([128, GB, FD], F8, tag="g", bufs=2)
                        nc.vector.memset(gz[:], 0)

                    def emit_half(ps, blk, h, first):
                        nch = ch_counts[blk][h]
                        base = ch_off[blk][h]
                        done = 0
                        for c0 in range(0, nch, GB):
                            gb = min(GB, nch - c0)
                            t0 = base + c0
                            g = apool.tile([128, GB, FD], F8, tag="g", bufs=2)
                            nc.gpsimd.dma_gather(
                                g[:, :gb, :], fulls[h][:],
                                idx_sb[:, t0 * 8:(t0 + gb) * 8],
                                gb * 128, gb * 128, FD,
                                single_packet=False)
                            for j in range(gb):
                                for nf in range(FD // 512):
                                    nc.tensor.matmul(
                                        ps[:, nf * 512:(nf + 1) * 512],
                                        s_sb[:, t0 + j, :],
                                        g[:, j, nf * 512:(nf + 1) * 512],
                                        start=(first and done == 0),
                                        stop=(done == nch - 1))
                                done += 1

                    # pass 1: self-loop + source-half-0 chunks for ALL bins,
                    # partials staged in SBUF.  Keeps the in-order gather
                    # queue busy during the half-1 AllGather instead of
                    # stalling at bin 0's half-1 gather.
                    accs = []
                    for blk in range(NB):
                        ps = aps.tile([128, FD], DT, tag="agg", bufs=1)
                        hb, rb = blk // (NB // 2), blk % (NB // 2)
                        sl = apool.tile([128, FD], mm_dt, tag="sl", bufs=2)
                        nc.sync.dma_start(
                            sl[:], locs[hb][rb * 128:(rb + 1) * 128, :])
                        for nf in range(FD // 512):
                            nc.tensor.matmul(
                                ps[:, nf * 512:(nf + 1) * 512],
                                id_sb[:],
                                sl[:, nf * 512:(nf + 1) * 512],
                                start=True, stop=False)
                        emit_half(ps, blk, 0, first=False)
                        acc = apool.tile([128, FD], mm_dt, tag=f"acc{blk}")
                        nc.vector.tensor_scalar_mul(
                            acc[:], ps[:], dv_sb[:, blk:blk + 1])
                        accs.append(acc)
                    # pass 2: half-1 chunks, combine with staged partials,
                    # transpose + tanh, then the fused consumer.
                    for blk in range(NB):
                        ps = aps.tile([128, FD], DT, tag="agg", bufs=1)
                        emit_half(ps, blk, 1, first=True)
                        a_nm = apool.tile([128, FD], mm_dt, tag="anm", bufs=2)
                        nc.vector.tensor_scalar_mul(
                            a_nm[:], ps[:], dv_sb[:, blk:blk + 1])
                        nc.vector.tensor_tensor(
                            out=a_nm[:], in0=a_nm[:], in1=accs[blk][:],
                            op=mybir.AluOpType.add)
                        for f in range(FD // 128):
                            pt = aps.tile([128, 128], mm_dt, tag="pt", bufs=2)
                            nc.tensor.transpose(
                                pt[:], a_nm[:, f * 128:(f + 1) * 128], id_sb[:])
                            nc.scalar.activation(
                                ht[:, f, blk * 128:(blk + 1) * 128], pt[:],
                                TANH, bias=b_sb[:, f:f + 1])
                        if consumer_fn:
                            consumer_fn(blk, state, apool, aps)

            for rep in range(repeat):
                with tc.tile_pool(name=pname("h1t"), bufs=1) as h1t_pool:
                    h1t = h1t_pool.tile([128, H1 // 128, R], mm_dt, tag="h1t")
                    for _ in range(pr["L1"]):
                        phase_L1(h1t)
                    for _ in range(pr["L2T"]):
                        transform(h1t, H1, H2, w2t_in, xw2_locs, xw2_locs8,
                                  xw2_fulls)
                with tc.tile_pool(name=pname("h2t"), bufs=1) as h2t_pool:
                    h2t = h2t_pool.tile([128, H2 // 128, R], mm_dt, tag="h2t")

                    def l3t_setup(apool):
                        wt3_sb = apool.tile([128, H2 // 128, H3], mm_dt,
                                            tag="wt3")
                        nc.sync.dma_start(wt3_sb[:], w3t_in[:])
                        return wt3_sb

                    def l3t_consumer(blk, wt3_sb, apool, aps):
                        """transform bin blk of h2t into xw3 rows; AllGather
                        each row half as soon as its 4 bins are done."""
                        ps3 = aps.tile([128, H3], DT, tag="xw3", bufs=1)
                        for k in range(H2 // 128):
                            for n0 in range(0, H3, 512):
                                n1 = min(n0 + 512, H3)
                                nc.tensor.matmul(
                                    ps3[:, n0:n1],
                                    h2t[:, k, blk * 128:(blk + 1) * 128],
                                    wt3_sb[:, k, n0:n1],
                                    start=(k == 0),
                                    stop=(k == H2 // 128 - 1))
                        o3 = apool.tile([128, H3], mm_dt, tag="o3", bufs=2)
                        nc.vector.tensor_scalar_mul(
                            o3[:], ps3[:], dv_sb[:, blk:blk + 1])
                        hb, rb = blk // (NB // 2), blk % (NB // 2)
                        nc.sync.dma_start(
                            xw3_locs[hb][rb * 128:(rb + 1) * 128, :], o3[:])
                        o38 = apool.tile([128, H3], F8, tag="o38", bufs=2)
                        nc.vector.tensor_scalar_mul(
                            o38[:], ps3[:], dv_sb[:, blk:blk + 1])
                        nc.sync.dma_start(
                            xw3_locs8[hb][rb * 128:(rb + 1) * 128, :], o38[:])
                        if rb == NB // 2 - 1:
                            if no_cc:
                                for cb in range(NUM_CORES):
                                    nc.sync.dma_start(
                                        xw3_fulls[hb][cb * RH:(cb + 1) * RH, :],
                                        xw3_locs8[hb][:])
                            else:
                                nc.gpsimd.collective_compute(
                                    "AllGather", mybir.AluOpType.bypass,
                                    replica_groups=[core_ids],
                                    ins=[xw3_locs8[hb][:]],
                                    outs=[xw3_fulls[hb][:]])

                    for _ in range(pr["L2A"]):
                        aggregate(xw2_fulls, xw2_locs, H2, h2t, b2_sb,
                                  setup_fn=l3t_setup,
                                  consumer_fn=l3t_consumer)
                with tc.tile_pool(name=pname("h3t"), bufs=1) as h3t_pool:
                    h3t = h3t_pool.tile([128, H3 // 128, R], mm_dt, tag="h3t")

                    def fin_setup(apool):
                        wlt_sb = apool.tile([128, H3 // 128, F_OUT], mm_dt,
                                            tag="wlt")
                        nc.sync.dma_start(wlt_sb[:], wlt_in[:])
                        return wlt_sb

                    def fin_consumer(blk, wlt_sb, apool, aps):
                        psf = aps.tile([128, F_OUT], DT, tag="xwf", bufs=1)
                        for k in range(H3 // 128):
                            for n0 in range(0, F_OUT, 512):
                                n1 = min(n0 + 512, F_OUT)
                                nc.tensor.matmul(
                                    psf[:, n0:n1],
                                    h3t[:, k, blk * 128:(blk + 1) * 128],
                                    wlt_sb[:, k, n0:n1],
                                    start=(k == 0),
                                    stop=(k == H3 // 128 - 1))
                        o = apool.tile([128, F_OUT], DT, tag="of", bufs=2)
                        nc.vector.tensor_tensor(
                            out=o[:], in0=psf[:], in1=bl_sb[:],
                            op=mybir.AluOpType.add)
                        nc.sync.dma_start(
                            out[blk * 128:(blk + 1) * 128, :], o[:])

                    for _ in range(pr["L3A"]):
                        aggregate(xw3_fulls, xw3_locs, H3, h3t, b3_sb,
                                  setup_fn=fin_setup,
                                  consumer_fn=fin_consumer)

    nc.compile()
    return nc


# ----------------------------------------------------------------------------
# Entry point
# ----------------------------------------------------------------------------

def _make_in_maps(inputs, perm, idx_tabs, s_mats, a_mats, dv_mats):
    import ml_dtypes
    mm_np = ml_dtypes.bfloat16 if MM_DT == mybir.dt.bfloat16 else np.float32
    f8_np = ml_dtypes.float8_e4m3

    def tile_w(w):  # [K, F] -> [128, K/128, F]
        k, f = w.shape
        return np.ascontiguousarray(
            w.reshape(k // 128, 128, f).transpose(1, 0, 2)).astype(mm_np)

    x_perm = np.ascontiguousarray(
        np.asarray(inputs["x"], np.float32)[perm]
        .reshape(N // 128, 128, F_IN).transpose(1, 0, 2)).astype(mm_np)
    w1t = tile_w(np.ascontiguousarray(np.asarray(inputs["W1"], np.float32).T))
    w2t = tile_w(np.ascontiguousarray(np.asarray(inputs["W2"], np.float32).T))
    w3t = tile_w(np.ascontiguousarray(np.asarray(inputs["W3"], np.float32).T))
    wlt = tile_w(np.ascontiguousarray(np.asarray(inputs["Wl"], np.float32).T))
    b1pp = np.ascontiguousarray(
        np.asarray(inputs["b1"], np.float32).reshape(-1, 128).T)
    b2pp = np.ascontiguousarray(
        np.asarray(inputs["b2"], np.float32).reshape(-1, 128).T)
    b3pp = np.ascontiguousarray(
        np.asarray(inputs["b3"], np.float32).reshape(-1, 128).T)
    blb = np.ascontiguousarray(
        np.broadcast_to(np.asarray(inputs["bl"], np.float32), (128, F_OUT)))
    ident = np.eye(128, dtype=mm_np)

    in_maps = []
    for c in range(NUM_CORES):
        in_maps.append({
            "x_perm": x_perm, "idx_in": idx_tabs[c],
            "s_in": s_mats[c].astype(f8_np),
            "a_in": a_mats[c].astype(mm_np),
            "dv_in": dv_mats[c],
            "ident": ident,
            "w1t": w1t, "w2t": w2t, "w3t": w3t, "wlt": wlt,
            "b1pp": b1pp, "b2pp": b2pp, "b3pp": b3pp, "blb": blb,
        })
    return in_maps


def _run(inputs, trace=False):
    perm, ch_counts, idx_tabs, s_mats, a_mats, dv_mats = _preprocess(
        np.asarray(inputs["edge_index"]))
    nc = _build_program(ch_counts, mm_dt=MM_DT)
    in_maps = _make_in_maps(inputs, perm, idx_tabs, s_mats, a_mats, dv_mats)
    res = run_bass_kernel_spmd(nc, in_maps, list(range(NUM_CORES)), trace=trace)
    out_perm = np.concatenate([res.results[c]["out"] for c in range(NUM_CORES)], 0)
    out = np.empty_like(out_perm)
    out[perm] = out_perm
    return out, res


def kernel(**inputs):
    out, _ = _run(inputs, trace=False)
    return out



# revision 76
# speedup vs baseline: 1.0780x; 1.0396x over previous
"""3-layer GCN + linear head on 8 Trainium2 NeuronCores.

Sharding: nodes are partitioned across the 8 cores (graph parallel), after a
host-side balanced permutation that gives every 128-node block exactly the
same number of incoming edges (including self loops).  Per layer each core:
  - transforms its 1024 rows (dense matmul, weights replicated),
  - AllGathers the transformed rows to every core,
  - gathers edge-source rows with SWDGE dma_gather and reduces them into
    destination rows with TensorE matmuls against host-built per-chunk
    selection matrices S (which carry the GCN edge normalization weights).
Layer 1 aggregates x first (256-dim) and transforms after, which is cheaper.
All arithmetic is fp32; accumulation in PSUM.
"""
import sys

if "/opt/trn_rl_repo" not in sys.path:
    sys.path.insert(0, "/opt/trn_rl_repo")

import numpy as np

import concourse.bass as bass
import concourse.mybir as mybir
import concourse.tile as tile
from concourse import bacc
from concourse.bass_utils import run_bass_kernel_spmd
from concourse.library_config import mlp

N = 8192
NUM_CORES = 8
R = N // NUM_CORES          # rows per core
NB = 8                      # dst blocks per core (128 rows each)
NBINS = NUM_CORES * NB
BIN_SZ = 128
F_IN, H1, H2, H3, F_OUT = 256, 2048, 2048, 1024, 768
DT = mybir.dt.float32
MM_DT = mybir.dt.bfloat16
F8 = mybir.dt.float8e4
TANH = mybir.ActivationFunctionType.Tanh


# ----------------------------------------------------------------------------
# Host-side graph preprocessing
# ----------------------------------------------------------------------------

def _preprocess(edge_index):
    src = np.asarray(edge_index[0], dtype=np.int64)
    dst = np.asarray(edge_index[1], dtype=np.int64)

    deg = np.bincount(dst, minlength=N).astype(np.float64) + 1.0
    dinv = 1.0 / np.sqrt(deg)
    d_in = np.bincount(dst, minlength=N) + 1

    # greedy balanced partition of nodes into bins of 128, equal in-edge sums
    order = np.argsort(-d_in, kind="stable")
    bin_sum = np.zeros(NBINS, dtype=np.int64)
    bin_cnt = np.zeros(NBINS, dtype=np.int64)
    bin_nodes = [[] for _ in range(NBINS)]
    for node in order:
        avail = np.where(bin_cnt < BIN_SZ)[0]
        b = avail[np.argmin(bin_sum[avail])]
        bin_nodes[b].append(node)
        bin_sum[b] += d_in[node]
        bin_cnt[b] += 1

    target = int(np.ceil(d_in.sum() / NBINS))
    for _ in range(200):
        hi = int(np.argmax(bin_sum))
        if bin_sum[hi] <= target:
            break
        lo = int(np.argmin(bin_sum))
        need = bin_sum[hi] - target
        best = None
        for ai, a in enumerate(bin_nodes[hi]):
            for bi, b in enumerate(bin_nodes[lo]):
                diff = d_in[a] - d_in[b]
                if diff > 0:
                    score = abs(diff - need)
                    if best is None or score < best[0]:
                        best = (score, ai, bi)
        if best is None:
            break
        _, ai, bi = best
        a, b = bin_nodes[hi][ai], bin_nodes[lo][bi]
        bin_nodes[hi][ai], bin_nodes[lo][bi] = b, a
        bin_sum[hi] += d_in[b] - d_in[a]
        bin_sum[lo] += d_in[a] - d_in[b]

    CH = int(np.ceil(bin_sum.max() / 128))
    EPB = CH * 128

    perm = np.concatenate([np.array(bn, dtype=np.int64) for bn in bin_nodes])
    inv = np.empty(N, dtype=np.int64)
    inv[perm] = np.arange(N)

    all_src = np.concatenate([inv[src], np.arange(N, dtype=np.int64)])
    all_dst = np.concatenate([inv[dst], np.arange(N, dtype=np.int64)])
    all_w = np.concatenate([
        (dinv[src] * dinv[dst]).astype(np.float32),
        (dinv[perm] * dinv[perm]).astype(np.float32),
    ])
    # self-loops (the appended tail) are handled as a per-bin diagonal matmul
    # against the core's own (local, pre-AllGather) rows — real edges only in
    # the gather tables.  Layer 1's dense-A keeps the full list.
    is_real = np.zeros(len(all_src), bool)
    is_real[:len(src)] = True
    sl_w = all_w[len(src):]          # dinv^2 per permuted node, in perm order

    bin_of = all_dst // BIN_SZ
    dst_local = all_dst % BIN_SZ

    NKC = N // 128                      # 64 source chunks for dense-A layer 1
    # Source rows are AllGathered in two row-halves per core: half h of core c
    # holds permuted rows [c*R + h*R/2, c*R + (h+1)*R/2), stored at row
    # c*R/2 + (local % (R/2)) of half-tensor h.  Aggregation chunks are
    # partitioned by source half so every dma_gather reads one tensor.
    RH = R // 2
    src_half = (all_src % R) // RH
    src_row = (all_src // R) * RH + (all_src % RH)

    # SPMD: one program for all cores, so chunk counts per (bin-slot, half)
    # must be uniform — take the max need across cores and pad.
    sels = [[np.where(bin_of == c * NB + blk)[0] for blk in range(NB)]
            for c in range(NUM_CORES)]
    rsels = [[s[is_real[s]] for s in row] for row in sels]
    ch_counts = []
    for blk in range(NB):
        nch = []
        for h in range(2):
            need = max(
                int(np.sum(src_half[rsels[c][blk]] == h))
                for c in range(NUM_CORES))
            nch.append(max(1, -(-need // 128)))
        ch_counts.append(tuple(nch))

    # layer-1 gather chunks: full-N source indices (x is an input, not
    # AllGathered, so no half split); real edges only, uniform counts
    ch1_counts = [
        max(1, -(-max(len(rsels[c][blk]) for c in range(NUM_CORES)) // 128))
        for blk in range(NB)]

    dinv_perm = dinv[perm].astype(np.float32)
    idx_tabs, s_mats, idx1_tabs, s1_mats, dv_mats = [], [], [], [], []
    for c in range(NUM_CORES):
        dv = np.empty((128, NB), np.float32)
        idx_cols, s_cols, idx1_cols, s1_cols = [], [], [], []
        for blk in range(NB):
            sel = rsels[c][blk]
            nodes = np.arange((c * NB + blk) * 128, (c * NB + blk + 1) * 128)
            dv[:, blk] = dinv_perm[nodes]
            for h in range(2):
                selh = sel[src_half[sel] == h]
                n_e = len(selh)
                n_ch = ch_counts[blk][h]
                epb = n_ch * 128
                e_src = np.zeros(epb, np.int64)
                e_dst = np.zeros(epb, np.int64)
                e_w = np.zeros(epb, np.float32)
                e_src[:n_e] = src_row[selh]
                e_dst[:n_e] = dst_local[selh]
                e_w[:n_e] = all_w[selh]
                ar = np.arange(epb)
                it = np.zeros((16, n_ch * 8), np.int16)
                it[ar % 16, ar // 16] = e_src.astype(np.int16)
                # edge weights are folded into the stored rows (dinv_src) and
                # the post-aggregation scale (dinv_dst): S is a 0/1 selector,
                # exact in fp8
                sm = np.zeros((128, n_ch, 128), np.float32)
                sm[ar % 128, ar // 128, e_dst] = (e_w != 0).astype(np.float32)
                idx_cols.append(it)
                s_cols.append(sm)
            # layer-1 tables: same 0/1 selector trick against the
            # dinv-prescaled x (full-N indices, single source tensor)
            n_e = len(sel)
            n_ch = ch1_counts[blk]
            epb = n_ch * 128
            e_src = np.zeros(epb, np.int64)
            e_dst = np.zeros(epb, np.int64)
            e_on = np.zeros(epb, np.float32)
            e_src[:n_e] = all_src[sel]
            e_dst[:n_e] = dst_local[sel]
            e_on[:n_e] = 1.0
            ar = np.arange(epb)
            it1 = np.zeros((16, n_ch * 8), np.int16)
            it1[ar % 16, ar // 16] = e_src.astype(np.int16)
            sm1 = np.zeros((128, n_ch, 128), np.float32)
            sm1[ar % 128, ar // 128, e_dst] = e_on
            idx1_cols.append(it1)
            s1_cols.append(sm1)
        idx_tabs.append(np.tile(np.concatenate(idx_cols, axis=1), (8, 1)))
        s_mats.append(np.concatenate(s_cols, axis=1))
        idx1_tabs.append(np.tile(np.concatenate(idx1_cols, axis=1), (8, 1)))
        s1_mats.append(np.concatenate(s1_cols, axis=1))
        dv_mats.append(dv)

    return (perm, dinv_perm, ch_counts, ch1_counts, idx_tabs, s_mats,
            idx1_tabs, s1_mats, dv_mats)


# ----------------------------------------------------------------------------
# Device program
# ----------------------------------------------------------------------------

def _build_program(ch_counts, ch1_counts, repeat=1, phase_reps=None, mm_dt=DT,
                   no_cc=False):
    """Build the SPMD program.  `repeat` repeats the whole pipeline (for
    timing); `phase_reps` maps phase name -> extra repetitions (timing only:
    a repeated phase recomputes the same values)."""
    pr = dict(L1=1, L2T=1, AG2=1, L2A=1, L3T=1, AG3=1, L3A=1, FIN=1)
    if phase_reps:
        pr.update(phase_reps)
    TOT_CH = sum(h0 + h1 for h0, h1 in ch_counts)
    # chunk-column offset of (blk, h) in the concatenated idx/s layout
    ch_off = []
    off = 0
    for h0, h1 in ch_counts:
        ch_off.append((off, off + h0))
        off += h0 + h1
    RH = R // 2
    nc = bacc.Bacc("TRN2", target_bir_lowering=False, debug=False,
                   num_devices=NUM_CORES)
    core_ids = list(range(NUM_CORES))

    T1 = sum(ch1_counts)
    ch1_off = [sum(ch1_counts[:b]) for b in range(NB)]
    # dinv-prescaled x: full copy (gather source) + this core's rows (self
    # loops, read locally like the locs tensors)
    x_full = nc.dram_tensor("x_full", [N, F_IN], mm_dt, kind="ExternalInput")
    x_loc = nc.dram_tensor("x_loc", [R, F_IN], mm_dt, kind="ExternalInput")
    idx_in = nc.dram_tensor("idx_in", [128, TOT_CH * 8], mybir.dt.int16,
                            kind="ExternalInput")
    s_in = nc.dram_tensor("s_in", [128, TOT_CH, 128], F8, kind="ExternalInput")
    idx1_in = nc.dram_tensor("idx1_in", [128, T1 * 8], mybir.dt.int16,
                             kind="ExternalInput")
    s1_in = nc.dram_tensor("s1_in", [128, T1, 128], mm_dt,
                           kind="ExternalInput")
    dv_in = nc.dram_tensor("dv_in", [128, NB], DT, kind="ExternalInput")
    ident_in = nc.dram_tensor("ident", [128, 128], mm_dt, kind="ExternalInput")
    # weights pre-tiled on host to [128, K/128, F] layout
    w1t_in = nc.dram_tensor("w1t", [128, F_IN // 128, H1], mm_dt, kind="ExternalInput")
    w2t_in = nc.dram_tensor("w2t", [128, H1 // 128, H2], mm_dt, kind="ExternalInput")
    w3t_in = nc.dram_tensor("w3t", [128, H2 // 128, H3], mm_dt, kind="ExternalInput")
    wlt_in = nc.dram_tensor("wlt", [128, H3 // 128, F_OUT], mm_dt, kind="ExternalInput")
    b1_in = nc.dram_tensor("b1pp", [128, H1 // 128], DT, kind="ExternalInput")
    b2_in = nc.dram_tensor("b2pp", [128, H2 // 128], DT, kind="ExternalInput")
    b3_in = nc.dram_tensor("b3pp", [128, H3 // 128], DT, kind="ExternalInput")
    bl_in = nc.dram_tensor("blb", [128, F_OUT], DT, kind="ExternalInput")

    out = nc.dram_tensor("out", [R, F_OUT], DT, kind="ExternalOutput")

    xw2_locs = [nc.dram_tensor(f"xw2_loc{h}", [RH, H2], mm_dt)
                for h in range(2)]
    xw2_locs8 = [nc.dram_tensor(f"xw2_loc8{h}", [RH, H2], F8)
                 for h in range(2)]
    xw2_fulls = [nc.dram_tensor(f"xw2_full{h}", [N // 2, H2], F8,
                                addr_space="Shared") for h in range(2)]
    xw3_locs = [nc.dram_tensor(f"xw3_loc{h}", [RH, H3], mm_dt)
                for h in range(2)]
    xw3_locs8 = [nc.dram_tensor(f"xw3_loc8{h}", [RH, H3], F8)
                 for h in range(2)]
    xw3_fulls = [nc.dram_tensor(f"xw3_full{h}", [N // 2, H3], F8,
                                addr_space="Shared") for h in range(2)]

    uid = [0]

    def pname(base):
        uid[0] += 1
        return f"{base}{uid[0]}"

    with tile.TileContext(nc) as tc:
        with tc.tile_pool(name="const", bufs=1) as cpool:
            nc.gpsimd.load_library(mlp)
            idx_sb = cpool.tile([128, TOT_CH * 8], mybir.dt.int16, tag="idx")
            nc.sync.dma_start(idx_sb[:], idx_in[:])
            s_sb = cpool.tile([128, TOT_CH, 128], F8, tag="s")
            nc.sync.dma_start(s_sb[:], s_in[:])
            dv_sb = cpool.tile([128, NB], DT, tag="dv")
            nc.sync.dma_start(dv_sb[:], dv_in[:])
            id_sb = cpool.tile([128, 128], mm_dt, tag="ident")
            nc.sync.dma_start(id_sb[:], ident_in[:])
            b1_sb = cpool.tile([128, H1 // 128], DT, tag="b1")
            nc.sync.dma_start(b1_sb[:], b1_in[:])
            b2_sb = cpool.tile([128, H2 // 128], DT, tag="b2")
            nc.sync.dma_start(b2_sb[:], b2_in[:])
            b3_sb = cpool.tile([128, H3 // 128], DT, tag="b3")
            nc.sync.dma_start(b3_sb[:], b3_in[:])
            bl_sb = cpool.tile([128, F_OUT], DT, tag="bl")
            nc.sync.dma_start(bl_sb[:], bl_in[:])

            def phase_L1(h1t):
                """gather dinv-prescaled x rows by edge source, 0/1-selector
                reduce + dinv_dst scale, transpose, transform."""
                with (
                    tc.tile_pool(name=pname("l1a"), bufs=1) as l1a_pool,
                    tc.tile_pool(name=pname("l1ps"), bufs=1, space="PSUM") as l1ps,
                ):
                    w1t_sb = l1a_pool.tile([128, F_IN // 128, H1], mm_dt, tag="w1t")
                    nc.sync.dma_start(w1t_sb[:], w1t_in[:])
                    idx1_sb = l1a_pool.tile([128, T1 * 8], mybir.dt.int16,
                                            tag="idx1")
                    nc.sync.dma_start(idx1_sb[:], idx1_in[:])
                    s1_sb = l1a_pool.tile([128, T1, 128], mm_dt, tag="s1")
                    nc.sync.dma_start(s1_sb[:], s1_in[:])
                    agg1t = l1a_pool.tile([128, F_IN // 128, R], mm_dt, tag="agg1t")
                    CH1M = max(ch1_counts)
                    for blk in range(NB):
                        nch = ch1_counts[blk]
                        t0 = ch1_off[blk]
                        ps = l1ps.tile([128, F_IN], DT, tag="agg", bufs=2)
                        sl = l1a_pool.tile([128, F_IN], mm_dt, tag="sl1",
                                           bufs=2)
                        nc.sync.dma_start(
                            sl[:], x_loc[blk * 128:(blk + 1) * 128, :])
                        nc.tensor.matmul(
                            ps[:, :], id_sb[:], sl[:, :],
                            start=True, stop=False)
                        g = l1a_pool.tile([128, CH1M, F_IN], mm_dt, tag="g1",
                                          bufs=2)
                        nc.gpsimd.dma_gather(
                            g[:, :nch, :], x_full[:],
                            idx1_sb[:, t0 * 8:(t0 + nch) * 8],
                            nch * 128, nch * 128, F_IN,
                            single_packet=False)
                        for j in range(nch):
                            nc.tensor.matmul(
                                ps[:, :], s1_sb[:, t0 + j, :], g[:, j, :],
                                start=False, stop=(j == nch - 1))
                        a_nm = l1a_pool.tile([128, F_IN], mm_dt, tag="anm", bufs=2)
                        nc.vector.tensor_scalar_mul(
                            a_nm[:], ps[:], dv_sb[:, blk:blk + 1])
                        for f in range(F_IN // 128):
                            pt = l1ps.tile([128, 128], mm_dt, tag="pt", bufs=2)
                            nc.tensor.transpose(
                                pt[:], a_nm[:, f * 128:(f + 1) * 128], id_sb[:])
                            nc.vector.tensor_copy(
                                agg1t[:, f, blk * 128:(blk + 1) * 128], pt[:])
                    for m in range(H1 // 128):
                        ps = l1ps.tile([128, R], DT, tag="xw", bufs=2)
                        for k in range(F_IN // 128):
                            for n in range(0, R, 512):
                                nc.tensor.matmul(
                                    ps[:, n:n + 512],
                                    w1t_sb[:, k, m * 128:(m + 1) * 128],
                                    agg1t[:, k, n:n + 512],
                                    start=(k == 0), stop=(k == F_IN // 128 - 1))
                        nc.scalar.activation(
                            h1t[:, m, :], ps[:], TANH, bias=b1_sb[:, m:m + 1])

            def transform(ht, KD, FD, wt_in, locs, locs8, fulls):
                """full-width (ht rows) @ W^T per 128-row block, node-major,
                pre-scaled by dinv of the row's node; written twice: bf16 for
                the local self-loop path and fp8 for the AllGather + gather
                path.  Each ROW half AllGathers as soon as its 4 blocks are
                written so the collective overlaps the other half's matmuls."""
                HK = KD // 128
                with (
                    tc.tile_pool(name=pname("tr"), bufs=1) as tpool,
                    tc.tile_pool(name=pname("trps"), bufs=1, space="PSUM") as tps,
                ):
                    wt_sb = tpool.tile([128, HK, FD], mm_dt, tag="wt")
                    nc.sync.dma_start(wt_sb[:], wt_in[:])
                    for h in range(2):
                        for r2 in range(NB // 2):
                            r = h * (NB // 2) + r2
                            ps = tps.tile([128, FD], DT, tag="xw", bufs=2)
                            for k in range(HK):
                                for n0 in range(0, FD, 512):
                                    n1 = min(n0 + 512, FD)
                                    nc.tensor.matmul(
                                        ps[:, n0:n1],
                                        ht[:, k, r * 128:(r + 1) * 128],
                                        wt_sb[:, k, n0:n1],
                                        start=(k == 0), stop=(k == HK - 1))
                            o = tpool.tile([128, FD], mm_dt, tag="o", bufs=3)
                            nc.vector.tensor_scalar_mul(
                                o[:], ps[:], dv_sb[:, r:r + 1])
                            nc.sync.dma_start(
                                locs[h][r2 * 128:(r2 + 1) * 128, :], o[:])
                            o8 = tpool.tile([128, FD], F8, tag="o8", bufs=3)
                            nc.vector.tensor_scalar_mul(
                                o8[:], ps[:], dv_sb[:, r:r + 1])
                            nc.sync.dma_start(
                                locs8[h][r2 * 128:(r2 + 1) * 128, :], o8[:])
                        if no_cc:
                            for cb in range(NUM_CORES):
                                nc.sync.dma_start(
                                    fulls[h][cb * RH:(cb + 1) * RH, :],
                                    locs8[h][:])
                        else:
                            nc.gpsimd.collective_compute(
                                "AllGather", mybir.AluOpType.bypass,
                                replica_groups=[core_ids],
                                ins=[locs8[h][:]], outs=[fulls[h][:]])

            def aggregate(fulls, locs, FD, ht, b_sb, setup_fn=None,
                          consumer_fn=None):
                """gather full-width rows by edge source (chunks partitioned
                by source row-half so each gather reads one contiguous
                AllGathered tensor), node-major reduce, transpose +
                tanh(.+bias) into feature-major ht.  Self-loops are a
                diagonal matmul against the core's own local rows (no
                collective dependency).  consumer_fn(blk, state, pools) emits
                the next phase's work for bin blk right after its activation,
                filling PE bubbles during later bins' gathers."""
                with (
                    tc.tile_pool(name=pname("ag"), bufs=1) as apool,
                    tc.tile_pool(name=pname("agps"), bufs=1, space="PSUM") as aps,
                ):
                    agg_bufs = 1 if FD * 4 // 512 >= 8 else 2
                    GB = max(1, 10240 // FD)
                    state = setup_fn(apool) if setup_fn else None
                    for i in range(2):
                        gz = apool.tile([128, GB, FD], F8, tag="g", bufs=2)
                        nc.vector.memset(gz[:], 0)
                    for blk in range(NB):
                        ps = aps.tile([128, FD], DT, tag="agg", bufs=agg_bufs)
                        hb, rb = blk // (NB // 2), blk % (NB // 2)
                        sl = apool.tile([128, FD], mm_dt, tag="sl", bufs=2)
                        nc.sync.dma_start(
                            sl[:], locs[hb][rb * 128:(rb + 1) * 128, :])
                        for nf in range(FD // 512):
                            nc.tensor.matmul(
                                ps[:, nf * 512:(nf + 1) * 512],
                                id_sb[:],
                                sl[:, nf * 512:(nf + 1) * 512],
                                start=True, stop=False)
                        tot = ch_counts[blk][0] + ch_counts[blk][1]
                        done = 0
                        for h in range(2):
                            nch = ch_counts[blk][h]
                            base = ch_off[blk][h]
                            for c0 in range(0, nch, GB):
                                gb = min(GB, nch - c0)
                                t0 = base + c0
                                g = apool.tile([128, GB, FD], F8,
                                               tag="g", bufs=2)
                                nc.gpsimd.dma_gather(
                                    g[:, :gb, :], fulls[h][:],
                                    idx_sb[:, t0 * 8:(t0 + gb) * 8],
                                    gb * 128, gb * 128, FD,
                                    single_packet=False)
                                for j in range(gb):
                                    for nf in range(FD // 512):
                                        nc.tensor.matmul(
                                            ps[:, nf * 512:(nf + 1) * 512],
                                            s_sb[:, t0 + j, :],
                                            g[:, j, nf * 512:(nf + 1) * 512],
                                            start=False,
                                            stop=(done == tot - 1))
                                    done += 1
                        a_nm = apool.tile([128, FD], mm_dt, tag="anm", bufs=2)
                        nc.vector.tensor_scalar_mul(
                            a_nm[:], ps[:], dv_sb[:, blk:blk + 1])
                        for f in range(FD // 128):
                            pt = aps.tile([128, 128], mm_dt, tag="pt", bufs=2)
                            nc.tensor.transpose(
                                pt[:], a_nm[:, f * 128:(f + 1) * 128], id_sb[:])
                            nc.scalar.activation(
                                ht[:, f, blk * 128:(blk + 1) * 128], pt[:],
                                TANH, bias=b_sb[:, f:f + 1])
                        if consumer_fn:
                            consumer_fn(blk, state, apool, aps)

            for rep in range(repeat):
                with tc.tile_pool(name=pname("h1t"), bufs=1) as h1t_pool:
                    h1t = h1t_pool.tile([128, H1 // 128, R], mm_dt, tag="h1t")
                    for _ in range(pr["L1"]):
                        phase_L1(h1t)
                    for _ in range(pr["L2T"]):
                        transform(h1t, H1, H2, w2t_in, xw2_locs, xw2_locs8,
                                  xw2_fulls)
                with tc.tile_pool(name=pname("h2t"), bufs=1) as h2t_pool:
                    h2t = h2t_pool.tile([128, H2 // 128, R], mm_dt, tag="h2t")

                    def l3t_setup(apool):
                        wt3_sb = apool.tile([128, H2 // 128, H3], mm_dt,
                                            tag="wt3")
                        nc.sync.dma_start(wt3_sb[:], w3t_in[:])
                        return wt3_sb

                    def l3t_consumer(blk, wt3_sb, apool, aps):
                        """transform bin blk of h2t into xw3 rows; AllGather
                        each row half as soon as its 4 bins are done."""
                        ps3 = aps.tile([128, H3], DT, tag="xw3", bufs=1)
                        for k in range(H2 // 128):
                            for n0 in range(0, H3, 512):
                                n1 = min(n0 + 512, H3)
                                nc.tensor.matmul(
                                    ps3[:, n0:n1],
                                    h2t[:, k, blk * 128:(blk + 1) * 128],
                                    wt3_sb[:, k, n0:n1],
                                    start=(k == 0),
                                    stop=(k == H2 // 128 - 1))
                        o3 = apool.tile([128, H3], mm_dt, tag="o3", bufs=2)
                        nc.vector.tensor_scalar_mul(
                            o3[:], ps3[:], dv_sb[:, blk:blk + 1])
                        hb, rb = blk // (NB // 2), blk % (NB // 2)
                        nc.sync.dma_start(
                            xw3_locs[hb][rb * 128:(rb + 1) * 128, :], o3[:])
                        o38 = apool.tile([128, H3], F8, tag="o38", bufs=2)
                        nc.vector.tensor_scalar_mul(
                            o38[:], ps3[:], dv_sb[:, blk:blk + 1])
                        nc.sync.dma_start(
                            xw3_locs8[hb][rb * 128:(rb + 1) * 128, :], o38[:])
                        if rb == NB // 2 - 1:
                            if no_cc:
                                for cb in range(NUM_CORES):
                                    nc.sync.dma_start(
                                        xw3_fulls[hb][cb * RH:(cb + 1) * RH, :],
                                        xw3_locs8[hb][:])
                            else:
                                nc.gpsimd.collective_compute(
                                    "AllGather", mybir.AluOpType.bypass,
                                    replica_groups=[core_ids],
                                    ins=[xw3_locs8[hb][:]],
                                    outs=[xw3_fulls[hb][:]])

                    for _ in range(pr["L2A"]):
                        aggregate(xw2_fulls, xw2_locs, H2, h2t, b2_sb,
                                  setup_fn=l3t_setup,
                                  consumer_fn=l3t_consumer)
                with tc.tile_pool(name=pname("h3t"), bufs=1) as h3t_pool:
                    h3t = h3t_pool.tile([128, H3 // 128, R], mm_dt, tag="h3t")

                    def fin_setup(apool):
                        wlt_sb = apool.tile([128, H3 // 128, F_OUT], mm_dt,
                                            tag="wlt")
                        nc.sync.dma_start(wlt_sb[:], wlt_in[:])
                        return wlt_sb

                    def fin_consumer(blk, wlt_sb, apool, aps):
                        psf = aps.tile([128, F_OUT], DT, tag="xwf", bufs=1)
                        for k in range(H3 // 128):
                            for n0 in range(0, F_OUT, 512):
                                n1 = min(n0 + 512, F_OUT)
                                nc.tensor.matmul(
                                    psf[:, n0:n1],
                                    h3t[:, k, blk * 128:(blk + 1) * 128],
                                    wlt_sb[:, k, n0:n1],
                                    start=(k == 0),
                                    stop=(k == H3 // 128 - 1))
                        o = apool.tile([128, F_OUT], DT, tag="of", bufs=2)
                        nc.vector.tensor_tensor(
                            out=o[:], in0=psf[:], in1=bl_sb[:],
                            op=mybir.AluOpType.add)
                        nc.sync.dma_start(
                            out[blk * 128:(blk + 1) * 128, :], o[:])

                    for _ in range(pr["L3A"]):
                        aggregate(xw3_fulls, xw3_locs, H3, h3t, b3_sb,
                                  setup_fn=fin_setup,
                                  consumer_fn=fin_consumer)

    nc.compile()
    return nc


# ----------------------------------------------------------------------------
# Entry point
# ----------------------------------------------------------------------------

def _make_in_maps(inputs, perm, dinv_perm, idx_tabs, s_mats, idx1_tabs,
                  s1_mats, dv_mats):
    import ml_dtypes
    mm_np = ml_dtypes.bfloat16 if MM_DT == mybir.dt.bfloat16 else np.float32
    f8_np = ml_dtypes.float8_e4m3

    def tile_w(w):  # [K, F] -> [128, K/128, F]
        k, f = w.shape
        return np.ascontiguousarray(
            w.reshape(k // 128, 128, f).transpose(1, 0, 2)).astype(mm_np)

    x_full = np.ascontiguousarray(
        dinv_perm[:, None] *
        np.asarray(inputs["x"], np.float32)[perm]).astype(mm_np)
    w1t = tile_w(np.ascontiguousarray(np.asarray(inputs["W1"], np.float32).T))
    w2t = tile_w(np.ascontiguousarray(np.asarray(inputs["W2"], np.float32).T))
    w3t = tile_w(np.ascontiguousarray(np.asarray(inputs["W3"], np.float32).T))
    wlt = tile_w(np.ascontiguousarray(np.asarray(inputs["Wl"], np.float32).T))
    b1pp = np.ascontiguousarray(
        np.asarray(inputs["b1"], np.float32).reshape(-1, 128).T)
    b2pp = np.ascontiguousarray(
        np.asarray(inputs["b2"], np.float32).reshape(-1, 128).T)
    b3pp = np.ascontiguousarray(
        np.asarray(inputs["b3"], np.float32).reshape(-1, 128).T)
    blb = np.ascontiguousarray(
        np.broadcast_to(np.asarray(inputs["bl"], np.float32), (128, F_OUT)))
    ident = np.eye(128, dtype=mm_np)

    in_maps = []
    for c in range(NUM_CORES):
        in_maps.append({
            "x_full": x_full,
            "x_loc": np.ascontiguousarray(x_full[c * R:(c + 1) * R]),
            "idx_in": idx_tabs[c],
            "s_in": s_mats[c].astype(f8_np),
            "idx1_in": idx1_tabs[c],
            "s1_in": s1_mats[c].astype(mm_np),
            "dv_in": dv_mats[c],
            "ident": ident,
            "w1t": w1t, "w2t": w2t, "w3t": w3t, "wlt": wlt,
            "b1pp": b1pp, "b2pp": b2pp, "b3pp": b3pp, "blb": blb,
        })
    return in_maps


def _run(inputs, trace=False):
    (perm, dinv_perm, ch_counts, ch1_counts, idx_tabs, s_mats, idx1_tabs,
     s1_mats, dv_mats) = _preprocess(np.asarray(inputs["edge_index"]))
    nc = _build_program(ch_counts, ch1_counts, mm_dt=MM_DT)
    in_maps = _make_in_maps(inputs, perm, dinv_perm, idx_tabs, s_mats,
                            idx1_tabs, s1_mats, dv_mats)
    res = run_bass_kernel_spmd(nc, in_maps, list(range(NUM_CORES)), trace=trace)
    out_perm = np.concatenate([res.results[c]["out"] for c in range(NUM_CORES)], 0)
    out = np.empty_like(out_perm)
    out[perm] = out_perm
    return out, res


def kernel(**inputs):
    out, _ = _run(inputs, trace=False)
    return out



# revision 86
# speedup vs baseline: 1.0908x; 1.0119x over previous
"""3-layer GCN + linear head on 8 Trainium2 NeuronCores.

Sharding: nodes are partitioned across the 8 cores (graph parallel), after a
host-side balanced permutation that gives every 128-node block exactly the
same number of incoming edges (including self loops).  Per layer each core:
  - transforms its 1024 rows (dense matmul, weights replicated),
  - AllGathers the transformed rows to every core,
  - gathers edge-source rows with SWDGE dma_gather and reduces them into
    destination rows with TensorE matmuls against host-built per-chunk
    selection matrices S (which carry the GCN edge normalization weights).
Layer 1 aggregates x first (256-dim) and transforms after, which is cheaper.
All arithmetic is fp32; accumulation in PSUM.
"""
import sys

if "/opt/trn_rl_repo" not in sys.path:
    sys.path.insert(0, "/opt/trn_rl_repo")

import numpy as np

import concourse.bass as bass
import concourse.mybir as mybir
import concourse.tile as tile
from concourse import bacc
from concourse.bass_utils import run_bass_kernel_spmd
from concourse.library_config import mlp

N = 8192
NUM_CORES = 8
R = N // NUM_CORES          # rows per core
NB = 8                      # dst blocks per core (128 rows each)
NBINS = NUM_CORES * NB
BIN_SZ = 128
F_IN, H1, H2, H3, F_OUT = 256, 2048, 2048, 1024, 768
DT = mybir.dt.float32
MM_DT = mybir.dt.bfloat16
F8 = mybir.dt.float8e4
TANH = mybir.ActivationFunctionType.Tanh


# ----------------------------------------------------------------------------
# Host-side graph preprocessing
# ----------------------------------------------------------------------------

def _preprocess(edge_index):
    src = np.asarray(edge_index[0], dtype=np.int64)
    dst = np.asarray(edge_index[1], dtype=np.int64)

    deg = np.bincount(dst, minlength=N).astype(np.float64) + 1.0
    dinv = 1.0 / np.sqrt(deg)
    d_in = np.bincount(dst, minlength=N) + 1

    # greedy balanced partition of nodes into bins of 128, equal in-edge sums
    order = np.argsort(-d_in, kind="stable")
    bin_sum = np.zeros(NBINS, dtype=np.int64)
    bin_cnt = np.zeros(NBINS, dtype=np.int64)
    bin_nodes = [[] for _ in range(NBINS)]
    for node in order:
        avail = np.where(bin_cnt < BIN_SZ)[0]
        b = avail[np.argmin(bin_sum[avail])]
        bin_nodes[b].append(node)
        bin_sum[b] += d_in[node]
        bin_cnt[b] += 1

    target = int(np.ceil(d_in.sum() / NBINS))
    for _ in range(200):
        hi = int(np.argmax(bin_sum))
        if bin_sum[hi] <= target:
            break
        lo = int(np.argmin(bin_sum))
        need = bin_sum[hi] - target
        best = None
        for ai, a in enumerate(bin_nodes[hi]):
            for bi, b in enumerate(bin_nodes[lo]):
                diff = d_in[a] - d_in[b]
                if diff > 0:
                    score = abs(diff - need)
                    if best is None or score < best[0]:
                        best = (score, ai, bi)
        if best is None:
            break
        _, ai, bi = best
        a, b = bin_nodes[hi][ai], bin_nodes[lo][bi]
        bin_nodes[hi][ai], bin_nodes[lo][bi] = b, a
        bin_sum[hi] += d_in[b] - d_in[a]
        bin_sum[lo] += d_in[a] - d_in[b]

    CH = int(np.ceil(bin_sum.max() / 128))
    EPB = CH * 128

    perm = np.concatenate([np.array(bn, dtype=np.int64) for bn in bin_nodes])
    inv = np.empty(N, dtype=np.int64)
    inv[perm] = np.arange(N)

    all_src = np.concatenate([inv[src], np.arange(N, dtype=np.int64)])
    all_dst = np.concatenate([inv[dst], np.arange(N, dtype=np.int64)])
    all_w = np.concatenate([
        (dinv[src] * dinv[dst]).astype(np.float32),
        (dinv[perm] * dinv[perm]).astype(np.float32),
    ])
    # self-loops (the appended tail) are handled as a per-bin diagonal matmul
    # against the core's own (local, pre-AllGather) rows — real edges only in
    # the gather tables.  Layer 1's dense-A keeps the full list.
    is_real = np.zeros(len(all_src), bool)
    is_real[:len(src)] = True
    sl_w = all_w[len(src):]          # dinv^2 per permuted node, in perm order

    bin_of = all_dst // BIN_SZ
    dst_local = all_dst % BIN_SZ

    NKC = N // 128                      # 64 source chunks for dense-A layer 1
    # Source rows are AllGathered in two row-halves per core: half h of core c
    # holds permuted rows [c*R + h*R/2, c*R + (h+1)*R/2), stored at row
    # c*R/2 + (local % (R/2)) of half-tensor h.  Aggregation chunks are
    # partitioned by source half so every dma_gather reads one tensor.
    RH = R // 2
    src_half = (all_src % R) // RH
    src_row = (all_src // R) * RH + (all_src % RH)

    # SPMD: one program for all cores, so chunk counts per (bin-slot, half)
    # must be uniform — take the max need across cores and pad.
    sels = [[np.where(bin_of == c * NB + blk)[0] for blk in range(NB)]
            for c in range(NUM_CORES)]
    rsels = [[s[is_real[s]] for s in row] for row in sels]
    ch_counts, need16 = [], []
    for blk in range(NB):
        nch, n16 = [], []
        for h in range(2):
            need = max(
                int(np.sum(src_half[rsels[c][blk]] == h))
                for c in range(NUM_CORES))
            nch.append(max(1, -(-need // 128)))
            # exact gather count (idx table wraps in 16s): skips transferring
            # most pad rows while the S-matmul still covers whole chunks
            n16.append(max(16, 16 * -(-need // 16)))
        ch_counts.append(tuple(nch))
        need16.append(tuple(n16))

    # layer-1 gather chunks: full-N source indices (x is an input, not
    # AllGathered, so no half split); real edges only, uniform counts
    ch1_counts, need16_1 = [], []
    for blk in range(NB):
        need = max(len(rsels[c][blk]) for c in range(NUM_CORES))
        ch1_counts.append(max(1, -(-need // 128)))
        need16_1.append(max(16, 16 * -(-need // 16)))

    dinv_perm = dinv[perm].astype(np.float32)
    idx_tabs, s_mats, idx1_tabs, s1_mats, dv_mats = [], [], [], [], []
    for c in range(NUM_CORES):
        dv = np.empty((128, NB), np.float32)
        idx_cols, s_cols, idx1_cols, s1_cols = [], [], [], []
        for blk in range(NB):
            sel = rsels[c][blk]
            nodes = np.arange((c * NB + blk) * 128, (c * NB + blk + 1) * 128)
            dv[:, blk] = dinv_perm[nodes]
            for h in range(2):
                selh = sel[src_half[sel] == h]
                n_e = len(selh)
                n_ch = ch_counts[blk][h]
                epb = n_ch * 128
                e_src = np.zeros(epb, np.int64)
                e_dst = np.zeros(epb, np.int64)
                e_w = np.zeros(epb, np.float32)
                e_src[:n_e] = src_row[selh]
                e_dst[:n_e] = dst_local[selh]
                e_w[:n_e] = all_w[selh]
                ar = np.arange(epb)
                it = np.zeros((16, n_ch * 8), np.int16)
                it[ar % 16, ar // 16] = e_src.astype(np.int16)
                # edge weights are folded into the stored rows (dinv_src) and
                # the post-aggregation scale (dinv_dst): S is a 0/1 selector,
                # exact in fp8
                sm = np.zeros((128, n_ch, 128), np.float32)
                sm[ar % 128, ar // 128, e_dst] = (e_w != 0).astype(np.float32)
                idx_cols.append(it)
                s_cols.append(sm)
            # layer-1 tables: same 0/1 selector trick against the
            # dinv-prescaled x (full-N indices, single source tensor)
            n_e = len(sel)
            n_ch = ch1_counts[blk]
            epb = n_ch * 128
            e_src = np.zeros(epb, np.int64)
            e_dst = np.zeros(epb, np.int64)
            e_on = np.zeros(epb, np.float32)
            e_src[:n_e] = all_src[sel]
            e_dst[:n_e] = dst_local[sel]
            e_on[:n_e] = 1.0
            ar = np.arange(epb)
            it1 = np.zeros((16, n_ch * 8), np.int16)
            it1[ar % 16, ar // 16] = e_src.astype(np.int16)
            sm1 = np.zeros((128, n_ch, 128), np.float32)
            sm1[ar % 128, ar // 128, e_dst] = e_on
            idx1_cols.append(it1)
            s1_cols.append(sm1)
        idx_tabs.append(np.tile(np.concatenate(idx_cols, axis=1), (8, 1)))
        s_mats.append(np.concatenate(s_cols, axis=1))
        idx1_tabs.append(np.tile(np.concatenate(idx1_cols, axis=1), (8, 1)))
        s1_mats.append(np.concatenate(s1_cols, axis=1))
        dv_mats.append(dv)

    return (perm, dinv_perm, ch_counts, ch1_counts, need16, need16_1,
            idx_tabs, s_mats, idx1_tabs, s1_mats, dv_mats)


# ----------------------------------------------------------------------------
# Device program
# ----------------------------------------------------------------------------

def _build_program(ch_counts, ch1_counts, need16, need16_1, repeat=1,
                   phase_reps=None, mm_dt=DT, no_cc=False):
    """Build the SPMD program.  `repeat` repeats the whole pipeline (for
    timing); `phase_reps` maps phase name -> extra repetitions (timing only:
    a repeated phase recomputes the same values)."""
    pr = dict(L1=1, L2T=1, AG2=1, L2A=1, L3T=1, AG3=1, L3A=1, FIN=1)
    if phase_reps:
        pr.update(phase_reps)
    TOT_CH = sum(h0 + h1 for h0, h1 in ch_counts)
    # chunk-column offset of (blk, h) in the concatenated idx/s layout
    ch_off = []
    off = 0
    for h0, h1 in ch_counts:
        ch_off.append((off, off + h0))
        off += h0 + h1
    RH = R // 2
    nc = bacc.Bacc("TRN2", target_bir_lowering=False, debug=False,
                   num_devices=NUM_CORES)
    core_ids = list(range(NUM_CORES))

    T1 = sum(ch1_counts)
    ch1_off = [sum(ch1_counts[:b]) for b in range(NB)]
    # dinv-prescaled x: full copy (gather source) + this core's rows (self
    # loops, read locally like the locs tensors)
    x_full = nc.dram_tensor("x_full", [N, F_IN], mm_dt, kind="ExternalInput")
    x_loc = nc.dram_tensor("x_loc", [R, F_IN], mm_dt, kind="ExternalInput")
    idx_in = nc.dram_tensor("idx_in", [128, TOT_CH * 8], mybir.dt.int16,
                            kind="ExternalInput")
    s_in = nc.dram_tensor("s_in", [128, TOT_CH, 128], F8, kind="ExternalInput")
    idx1_in = nc.dram_tensor("idx1_in", [128, T1 * 8], mybir.dt.int16,
                             kind="ExternalInput")
    s1_in = nc.dram_tensor("s1_in", [128, T1, 128], mm_dt,
                           kind="ExternalInput")
    dv_in = nc.dram_tensor("dv_in", [128, NB], DT, kind="ExternalInput")
    ident_in = nc.dram_tensor("ident", [128, 128], mm_dt, kind="ExternalInput")
    # weights pre-tiled on host to [128, K/128, F] layout
    w1t_in = nc.dram_tensor("w1t", [128, F_IN // 128, H1], mm_dt, kind="ExternalInput")
    w2t_in = nc.dram_tensor("w2t", [128, H1 // 128, H2], mm_dt, kind="ExternalInput")
    w3t_in = nc.dram_tensor("w3t", [128, H2 // 128, H3], mm_dt, kind="ExternalInput")
    wlt_in = nc.dram_tensor("wlt", [128, H3 // 128, F_OUT], mm_dt, kind="ExternalInput")
    b1_in = nc.dram_tensor("b1pp", [128, H1 // 128], DT, kind="ExternalInput")
    b2_in = nc.dram_tensor("b2pp", [128, H2 // 128], DT, kind="ExternalInput")
    b3_in = nc.dram_tensor("b3pp", [128, H3 // 128], DT, kind="ExternalInput")
    bl_in = nc.dram_tensor("blb", [128, F_OUT], DT, kind="ExternalInput")

    out = nc.dram_tensor("out", [R, F_OUT], DT, kind="ExternalOutput")

    xw2_locs = [nc.dram_tensor(f"xw2_loc{h}", [RH, H2], mm_dt)
                for h in range(2)]
    xw2_locs8 = [nc.dram_tensor(f"xw2_loc8{h}", [RH, H2], F8)
                 for h in range(2)]
    xw2_fulls = [nc.dram_tensor(f"xw2_full{h}", [N // 2, H2], F8,
                                addr_space="Shared") for h in range(2)]
    xw3_locs = [nc.dram_tensor(f"xw3_loc{h}", [RH, H3], mm_dt)
                for h in range(2)]
    xw3_locs8 = [nc.dram_tensor(f"xw3_loc8{h}", [RH, H3], F8)
                 for h in range(2)]
    xw3_fulls = [nc.dram_tensor(f"xw3_full{h}", [N // 2, H3], F8,
                                addr_space="Shared") for h in range(2)]

    uid = [0]

    def pname(base):
        uid[0] += 1
        return f"{base}{uid[0]}"

    with tile.TileContext(nc) as tc:
        with tc.tile_pool(name="const", bufs=1) as cpool:
            nc.gpsimd.load_library(mlp)
            idx_sb = cpool.tile([128, TOT_CH * 8], mybir.dt.int16, tag="idx")
            nc.sync.dma_start(idx_sb[:], idx_in[:])
            s_sb = cpool.tile([128, TOT_CH, 128], F8, tag="s")
            nc.sync.dma_start(s_sb[:], s_in[:])
            dv_sb = cpool.tile([128, NB], DT, tag="dv")
            nc.sync.dma_start(dv_sb[:], dv_in[:])
            id_sb = cpool.tile([128, 128], mm_dt, tag="ident")
            nc.sync.dma_start(id_sb[:], ident_in[:])
            b1_sb = cpool.tile([128, H1 // 128], DT, tag="b1")
            nc.sync.dma_start(b1_sb[:], b1_in[:])
            b2_sb = cpool.tile([128, H2 // 128], DT, tag="b2")
            nc.sync.dma_start(b2_sb[:], b2_in[:])
            b3_sb = cpool.tile([128, H3 // 128], DT, tag="b3")
            nc.sync.dma_start(b3_sb[:], b3_in[:])
            bl_sb = cpool.tile([128, F_OUT], DT, tag="bl")
            nc.sync.dma_start(bl_sb[:], bl_in[:])

            def phase_L1(h1t):
                """gather dinv-prescaled x rows by edge source, 0/1-selector
                reduce + dinv_dst scale, transpose, transform."""
                with (
                    tc.tile_pool(name=pname("l1a"), bufs=1) as l1a_pool,
                    tc.tile_pool(name=pname("l1ps"), bufs=1, space="PSUM") as l1ps,
                ):
                    w1t_sb = l1a_pool.tile([128, F_IN // 128, H1], mm_dt, tag="w1t")
                    nc.sync.dma_start(w1t_sb[:], w1t_in[:])
                    idx1_sb = l1a_pool.tile([128, T1 * 8], mybir.dt.int16,
                                            tag="idx1")
                    nc.sync.dma_start(idx1_sb[:], idx1_in[:])
                    s1_sb = l1a_pool.tile([128, T1, 128], mm_dt, tag="s1")
                    nc.sync.dma_start(s1_sb[:], s1_in[:])
                    agg1t = l1a_pool.tile([128, F_IN // 128, R], mm_dt, tag="agg1t")
                    CH1M = max(ch1_counts)
                    for blk in range(NB):
                        nch = ch1_counts[blk]
                        t0 = ch1_off[blk]
                        ps = l1ps.tile([128, F_IN], DT, tag="agg", bufs=2)
                        sl = l1a_pool.tile([128, F_IN], mm_dt, tag="sl1",
                                           bufs=2)
                        nc.sync.dma_start(
                            sl[:], x_loc[blk * 128:(blk + 1) * 128, :])
                        nc.tensor.matmul(
                            ps[:, :], id_sb[:], sl[:, :],
                            start=True, stop=False)
                        g = l1a_pool.tile([128, CH1M, F_IN], mm_dt, tag="g1",
                                          bufs=2)
                        nid = need16_1[blk]
                        nc.gpsimd.dma_gather(
                            g[:, :nch, :], x_full[:],
                            idx1_sb[:, t0 * 8:t0 * 8 + nid // 16],
                            nid, nid, F_IN,
                            single_packet=False)
                        for j in range(nch):
                            nc.tensor.matmul(
                                ps[:, :], s1_sb[:, t0 + j, :], g[:, j, :],
                                start=False, stop=(j == nch - 1))
                        a_nm = l1a_pool.tile([128, F_IN], mm_dt, tag="anm", bufs=2)
                        nc.vector.tensor_scalar_mul(
                            a_nm[:], ps[:], dv_sb[:, blk:blk + 1])
                        for f in range(F_IN // 128):
                            pt = l1ps.tile([128, 128], mm_dt, tag="pt", bufs=2)
                            nc.tensor.transpose(
                                pt[:], a_nm[:, f * 128:(f + 1) * 128], id_sb[:])
                            nc.vector.tensor_copy(
                                agg1t[:, f, blk * 128:(blk + 1) * 128], pt[:])
                    for m in range(H1 // 128):
                        ps = l1ps.tile([128, R], DT, tag="xw", bufs=2)
                        for k in range(F_IN // 128):
                            for n in range(0, R, 512):
                                nc.tensor.matmul(
                                    ps[:, n:n + 512],
                                    w1t_sb[:, k, m * 128:(m + 1) * 128],
                                    agg1t[:, k, n:n + 512],
                                    start=(k == 0), stop=(k == F_IN // 128 - 1))
                        nc.scalar.activation(
                            h1t[:, m, :], ps[:], TANH, bias=b1_sb[:, m:m + 1])

            def transform(ht, KD, FD, wt_in, locs, locs8, fulls):
                """full-width (ht rows) @ W^T per 128-row block, node-major,
                pre-scaled by dinv of the row's node; written twice: bf16 for
                the local self-loop path and fp8 for the AllGather + gather
                path.  Each ROW half AllGathers as soon as its 4 blocks are
                written so the collective overlaps the other half's matmuls."""
                HK = KD // 128
                with (
                    tc.tile_pool(name=pname("tr"), bufs=1) as tpool,
                    tc.tile_pool(name=pname("trps"), bufs=1, space="PSUM") as tps,
                ):
                    wt_sb = tpool.tile([128, HK, FD], mm_dt, tag="wt")
                    nc.sync.dma_start(wt_sb[:], wt_in[:])
                    for h in range(2):
                        for r2 in range(NB // 2):
                            r = h * (NB // 2) + r2
                            ps = tps.tile([128, FD], DT, tag="xw", bufs=2)
                            for k in range(HK):
                                for n0 in range(0, FD, 512):
                                    n1 = min(n0 + 512, FD)
                                    nc.tensor.matmul(
                                        ps[:, n0:n1],
                                        ht[:, k, r * 128:(r + 1) * 128],
                                        wt_sb[:, k, n0:n1],
                                        start=(k == 0), stop=(k == HK - 1))
                            o = tpool.tile([128, FD], mm_dt, tag="o", bufs=3)
                            nc.vector.tensor_scalar_mul(
                                o[:], ps[:], dv_sb[:, r:r + 1])
                            nc.sync.dma_start(
                                locs[h][r2 * 128:(r2 + 1) * 128, :], o[:])
                            o8 = tpool.tile([128, FD], F8, tag="o8", bufs=3)
                            nc.vector.tensor_scalar_mul(
                                o8[:], ps[:], dv_sb[:, r:r + 1])
                            nc.sync.dma_start(
                                locs8[h][r2 * 128:(r2 + 1) * 128, :], o8[:])
                        if no_cc:
                            for cb in range(NUM_CORES):
                                nc.sync.dma_start(
                                    fulls[h][cb * RH:(cb + 1) * RH, :],
                                    locs8[h][:])
                        else:
                            nc.gpsimd.collective_compute(
                                "AllGather", mybir.AluOpType.bypass,
                                replica_groups=[core_ids],
                                ins=[locs8[h][:]], outs=[fulls[h][:]])

            def aggregate(fulls, locs, FD, ht, b_sb, setup_fn=None,
                          consumer_fn=None):
                """gather full-width rows by edge source (chunks partitioned
                by source row-half so each gather reads one contiguous
                AllGathered tensor), node-major reduce, transpose +
                tanh(.+bias) into feature-major ht.  Self-loops are a
                diagonal matmul against the core's own local rows (no
                collective dependency).  consumer_fn(blk, state, pools) emits
                the next phase's work for bin blk right after its activation,
                filling PE bubbles during later bins' gathers."""
                with (
                    tc.tile_pool(name=pname("ag"), bufs=1) as apool,
                    tc.tile_pool(name=pname("agps"), bufs=1, space="PSUM") as aps,
                ):
                    agg_bufs = 1 if FD * 4 // 512 >= 8 else 2
                    GB = max(1, 10240 // FD)
                    state = setup_fn(apool) if setup_fn else None
                    for i in range(2):
                        gz = apool.tile([128, GB, FD], F8, tag="g", bufs=2)
                        nc.vector.memset(gz[:], 0)
                    for blk in range(NB):
                        ps = aps.tile([128, FD], DT, tag="agg", bufs=agg_bufs)
                        hb, rb = blk // (NB // 2), blk % (NB // 2)
                        sl = apool.tile([128, FD], mm_dt, tag="sl", bufs=2)
                        nc.sync.dma_start(
                            sl[:], locs[hb][rb * 128:(rb + 1) * 128, :])
                        for nf in range(FD // 512):
                            nc.tensor.matmul(
                                ps[:, nf * 512:(nf + 1) * 512],
                                id_sb[:],
                                sl[:, nf * 512:(nf + 1) * 512],
                                start=True, stop=False)
                        tot = ch_counts[blk][0] + ch_counts[blk][1]
                        done = 0
                        for h in range(2):
                            nch = ch_counts[blk][h]
                            base = ch_off[blk][h]
                            for c0 in range(0, nch, GB):
                                gb = min(GB, nch - c0)
                                t0 = base + c0
                                nid = (need16[blk][h] - c0 * 128
                                       if c0 + gb == nch else gb * 128)
                                g = apool.tile([128, GB, FD], F8,
                                               tag="g", bufs=2)
                                nc.gpsimd.dma_gather(
                                    g[:, :gb, :], fulls[h][:],
                                    idx_sb[:, t0 * 8:t0 * 8 + nid // 16],
                                    nid, nid, FD,
                                    single_packet=False)
                                for j in range(gb):
                                    for nf in range(FD // 512):
                                        nc.tensor.matmul(
                                            ps[:, nf * 512:(nf + 1) * 512],
                                            s_sb[:, t0 + j, :],
                                            g[:, j, nf * 512:(nf + 1) * 512],
                                            start=False,
                                            stop=(done == tot - 1))
                                    done += 1
                        a_nm = apool.tile([128, FD], mm_dt, tag="anm", bufs=2)
                        nc.vector.tensor_scalar_mul(
                            a_nm[:], ps[:], dv_sb[:, blk:blk + 1])
                        for f in range(FD // 128):
                            pt = aps.tile([128, 128], mm_dt, tag="pt", bufs=2)
                            nc.tensor.transpose(
                                pt[:], a_nm[:, f * 128:(f + 1) * 128], id_sb[:])
                            nc.scalar.activation(
                                ht[:, f, blk * 128:(blk + 1) * 128], pt[:],
                                TANH, bias=b_sb[:, f:f + 1])
                        if consumer_fn:
                            consumer_fn(blk, state, apool, aps)

            for rep in range(repeat):
                with tc.tile_pool(name=pname("h1t"), bufs=1) as h1t_pool:
                    h1t = h1t_pool.tile([128, H1 // 128, R], mm_dt, tag="h1t")
                    for _ in range(pr["L1"]):
                        phase_L1(h1t)
                    for _ in range(pr["L2T"]):
                        transform(h1t, H1, H2, w2t_in, xw2_locs, xw2_locs8,
                                  xw2_fulls)
                with tc.tile_pool(name=pname("h2t"), bufs=1) as h2t_pool:
                    h2t = h2t_pool.tile([128, H2 // 128, R], mm_dt, tag="h2t")

                    def l3t_setup(apool):
                        wt3_sb = apool.tile([128, H2 // 128, H3], mm_dt,
                                            tag="wt3")
                        nc.sync.dma_start(wt3_sb[:], w3t_in[:])
                        return wt3_sb

                    def l3t_consumer(blk, wt3_sb, apool, aps):
                        """transform bin blk of h2t into xw3 rows; AllGather
                        each row half as soon as its 4 bins are done."""
                        ps3 = aps.tile([128, H3], DT, tag="xw3", bufs=1)
                        for k in range(H2 // 128):
                            for n0 in range(0, H3, 512):
                                n1 = min(n0 + 512, H3)
                                nc.tensor.matmul(
                                    ps3[:, n0:n1],
                                    h2t[:, k, blk * 128:(blk + 1) * 128],
                                    wt3_sb[:, k, n0:n1],
                                    start=(k == 0),
                                    stop=(k == H2 // 128 - 1))
                        o3 = apool.tile([128, H3], mm_dt, tag="o3", bufs=2)
                        nc.vector.tensor_scalar_mul(
                            o3[:], ps3[:], dv_sb[:, blk:blk + 1])
                        hb, rb = blk // (NB // 2), blk % (NB // 2)
                        nc.sync.dma_start(
                            xw3_locs[hb][rb * 128:(rb + 1) * 128, :], o3[:])
                        o38 = apool.tile([128, H3], F8, tag="o38", bufs=2)
                        nc.vector.tensor_scalar_mul(
                            o38[:], ps3[:], dv_sb[:, blk:blk + 1])
                        nc.sync.dma_start(
                            xw3_locs8[hb][rb * 128:(rb + 1) * 128, :], o38[:])
                        if rb == NB // 2 - 1:
                            if no_cc:
                                for cb in range(NUM_CORES):
                                    nc.sync.dma_start(
                                        xw3_fulls[hb][cb * RH:(cb + 1) * RH, :],
                                        xw3_locs8[hb][:])
                            else:
                                nc.gpsimd.collective_compute(
                                    "AllGather", mybir.AluOpType.bypass,
                                    replica_groups=[core_ids],
                                    ins=[xw3_locs8[hb][:]],
                                    outs=[xw3_fulls[hb][:]])

                    for _ in range(pr["L2A"]):
                        aggregate(xw2_fulls, xw2_locs, H2, h2t, b2_sb,
                                  setup_fn=l3t_setup,
                                  consumer_fn=l3t_consumer)
                with tc.tile_pool(name=pname("h3t"), bufs=1) as h3t_pool:
                    h3t = h3t_pool.tile([128, H3 // 128, R], mm_dt, tag="h3t")

                    def fin_setup(apool):
                        wlt_sb = apool.tile([128, H3 // 128, F_OUT], mm_dt,
                                            tag="wlt")
                        nc.sync.dma_start(wlt_sb[:], wlt_in[:])
                        return wlt_sb

                    def fin_consumer(blk, wlt_sb, apool, aps):
                        psf = aps.tile([128, F_OUT], DT, tag="xwf", bufs=1)
                        for k in range(H3 // 128):
                            for n0 in range(0, F_OUT, 512):
                                n1 = min(n0 + 512, F_OUT)
                                nc.tensor.matmul(
                                    psf[:, n0:n1],
                                    h3t[:, k, blk * 128:(blk + 1) * 128],
                                    wlt_sb[:, k, n0:n1],
                                    start=(k == 0),
                                    stop=(k == H3 // 128 - 1))
                        o = apool.tile([128, F_OUT], DT, tag="of", bufs=2)
                        nc.vector.tensor_tensor(
                            out=o[:], in0=psf[:], in1=bl_sb[:],
                            op=mybir.AluOpType.add)
                        nc.sync.dma_start(
                            out[blk * 128:(blk + 1) * 128, :], o[:])

                    for _ in range(pr["L3A"]):
                        aggregate(xw3_fulls, xw3_locs, H3, h3t, b3_sb,
                                  setup_fn=fin_setup,
                                  consumer_fn=fin_consumer)

    nc.compile()
    return nc


# ----------------------------------------------------------------------------
# Entry point
# ----------------------------------------------------------------------------

def _make_in_maps(inputs, perm, dinv_perm, idx_tabs, s_mats, idx1_tabs,
                  s1_mats, dv_mats):
    import ml_dtypes
    mm_np = ml_dtypes.bfloat16 if MM_DT == mybir.dt.bfloat16 else np.float32
    f8_np = ml_dtypes.float8_e4m3

    def tile_w(w):  # [K, F] -> [128, K/128, F]
        k, f = w.shape
        return np.ascontiguousarray(
            w.reshape(k // 128, 128, f).transpose(1, 0, 2)).astype(mm_np)

    x_full = np.ascontiguousarray(
        dinv_perm[:, None] *
        np.asarray(inputs["x"], np.float32)[perm]).astype(mm_np)
    w1t = tile_w(np.ascontiguousarray(np.asarray(inputs["W1"], np.float32).T))
    w2t = tile_w(np.ascontiguousarray(np.asarray(inputs["W2"], np.float32).T))
    w3t = tile_w(np.ascontiguousarray(np.asarray(inputs["W3"], np.float32).T))
    wlt = tile_w(np.ascontiguousarray(np.asarray(inputs["Wl"], np.float32).T))
    b1pp = np.ascontiguousarray(
        np.asarray(inputs["b1"], np.float32).reshape(-1, 128).T)
    b2pp = np.ascontiguousarray(
        np.asarray(inputs["b2"], np.float32).reshape(-1, 128).T)
    b3pp = np.ascontiguousarray(
        np.asarray(inputs["b3"], np.float32).reshape(-1, 128).T)
    blb = np.ascontiguousarray(
        np.broadcast_to(np.asarray(inputs["bl"], np.float32), (128, F_OUT)))
    ident = np.eye(128, dtype=mm_np)

    in_maps = []
    for c in range(NUM_CORES):
        in_maps.append({
            "x_full": x_full,
            "x_loc": np.ascontiguousarray(x_full[c * R:(c + 1) * R]),
            "idx_in": idx_tabs[c],
            "s_in": s_mats[c].astype(f8_np),
            "idx1_in": idx1_tabs[c],
            "s1_in": s1_mats[c].astype(mm_np),
            "dv_in": dv_mats[c],
            "ident": ident,
            "w1t": w1t, "w2t": w2t, "w3t": w3t, "wlt": wlt,
            "b1pp": b1pp, "b2pp": b2pp, "b3pp": b3pp, "blb": blb,
        })
    return in_maps


def _run(inputs, trace=False):
    (perm, dinv_perm, ch_counts, ch1_counts, need16, need16_1, idx_tabs,
     s_mats, idx1_tabs, s1_mats, dv_mats) = _preprocess(
        np.asarray(inputs["edge_index"]))
    nc = _build_program(ch_counts, ch1_counts, need16, need16_1, mm_dt=MM_DT)
    in_maps = _make_in_maps(inputs, perm, dinv_perm, idx_tabs, s_mats,
                            idx1_tabs, s1_mats, dv_mats)
    res = run_bass_kernel_spmd(nc, in_maps, list(range(NUM_CORES)), trace=trace)
    out_perm = np.concatenate([res.results[c]["out"] for c in range(NUM_CORES)], 0)
    out = np.empty_like(out_perm)
    out[perm] = out_perm
    return out, res


def kernel(**inputs):
    out, _ = _run(inputs, trace=False)
    return out

